# revision 11
# baseline (speedup 1.0000x reference)
"""Area attention (B=64, L=512, D=256, W=3) on 8 TRN2 NeuronCores.

Data parallel over batch: 8 batches per core, processed in pairs so the
elementwise work runs as merged [128, 2, N] instructions (half the
instruction-count overhead). Inputs are cast to f16 on the host (the device
kernel always computed in f16; shipping f16 halves HBM traffic and removes
the on-device casts), and q/k are loaded pre-transposed straight from DRAM
via the XBAR dma-transpose path, which removes all PE input transposes and
their PSUM->SBUF copy-backs.

Per (row-block j, batch pair) the kernel builds the three area-width score
strips [s0, ss1/2, ss2/3] in one SBUF tile so a SINGLE exp activation per
batch (scale 1/T) covers all widths; width scales ride a 4x-mode
tensor_scalar. The softmax denominator is recovered by a 4x tensor_scalar
bypass-copy with accum_out over u = p0 + (p1+p2) (whose row sum IS the
denominator), so no activation accumulator reads are needed. Width pooling
of the values folds into shifted sums of the probabilities (Qtot), keeping
the output matmul contraction at L rather than L*W; 1/rowsum is applied to
Qtot before the PE transpose, so the output matmul result is final and is
DMA'd to DRAM directly from PSUM.
"""

import numpy as np

B, L, D = 64, 512, 256
W = 3
NCORES = 8
NB = B // NCORES  # batches per core
P = 128
RB = L // P  # 4 row blocks of 128
DB = D // P  # 2 contraction blocks of 128
NEG = -30000.0
TEMP = float(np.sqrt(D))  # 16.0
XW = 520  # strip pitch (>= L + 2 pad)

TRACE = False
LAST_EXEC_NS = None
LAST_RESULTS = None

_NC_CACHE = {}


def _build_nc(for_sim=False):
    from contextlib import ExitStack

    import concourse.bacc as bacc
    import concourse.bass as bass
    import concourse.tile as tile
    from concourse import mybir

    f32 = mybir.dt.float32
    f16 = mybir.dt.float16
    EXP = mybir.ActivationFunctionType.Exp
    ADD = mybir.AluOpType.add
    MULT = mybir.AluOpType.mult

    if for_sim:
        nc = bass.Bass()
    else:
        nc = bacc.Bacc(None, target_bir_lowering=False)

    q_ext = nc.declare_dram_parameter("q", [NB, L, D], f16, isOutput=False)
    k_ext = nc.declare_dram_parameter("key", [NB, L, D], f16, isOutput=False)
    v_ext = nc.declare_dram_parameter("val", [NB, L, D], f16, isOutput=False)
    nm16_ext = nc.declare_dram_parameter("nm16", [P, P], f16, isOutput=False)
    id16_ext = nc.declare_dram_parameter("id16", [P, P], f16, isOutput=False)
    out_ext = nc.declare_dram_parameter("out", [NB, L, D], f16, isOutput=True)

    with tile.TileContext(nc) as tc, ExitStack() as ctx:
        const = ctx.enter_context(tc.tile_pool(name="const", bufs=1))
        nm16 = const.tile([P, P], f16)
        id16 = const.tile([P, P], f16)
        warm = const.tile([P, 1], f16)
        nc.gpsimd.dma_start(out=nm16[:], in_=nm16_ext[:])
        nc.gpsimd.dma_start(out=id16[:], in_=id16_ext[:])
        # warm the Exp activation table while the first loads are in flight
        nc.vector.memset(warm[:], 0.0)
        nc.scalar.activation(warm[:], warm[:], EXP, scale=1.0)

        inpool = ctx.enter_context(tc.tile_pool(name="inpool", bufs=2))
        xpool = ctx.enter_context(tc.tile_pool(name="xpool", bufs=2))
        papool = ctx.enter_context(tc.tile_pool(name="papool", bufs=2))
        agpool = ctx.enter_context(tc.tile_pool(name="agpool", bufs=2))
        qtpool = ctx.enter_context(tc.tile_pool(name="qtpool", bufs=2))
        opool = ctx.enter_context(tc.tile_pool(name="opool", bufs=2))
        rspool = ctx.enter_context(tc.tile_pool(name="rspool", bufs=2))
        psum_s0 = ctx.enter_context(tc.tile_pool(name="psum_s0", bufs=1, space="PSUM"))
        psum_tr = ctx.enter_context(tc.tile_pool(name="psum_tr", bufs=2, space="PSUM"))
        psum_o = ctx.enter_context(tc.tile_pool(name="psum_o", bufs=1, space="PSUM"))

        for bp in range(NB // 2):
            qTs, kTs, vs, outps = [], [], [], []
            for bb in range(2):
                b = 2 * bp + bb
                qT = inpool.tile([P, DB, L], f16, tag=f"qT{bb}")
                kT = inpool.tile([P, DB, L], f16, tag=f"kT{bb}")
                v16 = inpool.tile([P, RB, D], f16, tag=f"v{bb}")
                nc.sync.dma_start_transpose(out=qT[:], in_=q_ext[b])
                nc.sync.dma_start_transpose(out=kT[:], in_=k_ext[b])
                nc.sync.dma_start(out=v16[:], in_=v_ext[b].rearrange("(r p) d -> p r d", p=P))
                qTs.append(qT)
                kTs.append(kT)
                vs.append(v16)
                outp_a = psum_o.tile([P, 2, D], f32, tag=f"outpa{bb}")
                outp_b = psum_o.tile([P, 2, D], f32, tag=f"outpb{bb}")
                outps.append((outp_a, outp_b))

            for j in range(RB):
                N = P * (j + 1)
                Np = N + 2
                # scores for both batches (f32: matmul output requirement)
                s0p = psum_s0.tile([P, 2, L], f32, tag="s0p")
                for bb in range(2):
                    for dblk in range(DB):
                        nc.tensor.matmul(
                            s0p[:, bb, 0:N],
                            qTs[bb][:, dblk, j * P:(j + 1) * P],
                            kTs[bb][:, dblk, 0:N],
                            start=(dblk == 0),
                            stop=False,
                        )
                    # causal mask accumulated on the PE (identity-stationary)
                    nc.tensor.matmul(
                        s0p[:, bb, j * P:N], id16[:], nm16[:], start=False, stop=True
                    )

                # strip tile: X[:, b, w, t]; w=0 raw s0, w=1 ss1/2, w=2 ss2/3
                X = xpool.tile([P, 2, W, XW], f16, tag="X")
                nc.gpsimd.memset(X[:, :, 1:3, 0:2], NEG)
                nc.gpsimd.memset(X[:, :, :, N:Np], NEG)
                nc.vector.tensor_copy(out=X[:, :, 0, 0:N], in_=s0p[:, :, 0:N])
                # Pool can't read PSUM: width-2 sum reads the f16 copy
                nc.gpsimd.tensor_add(
                    X[:, :, 1, 1:N], X[:, :, 0, 1:N], X[:, :, 0, 0:N - 1]
                )
                nc.vector.tensor_add(
                    X[:, :, 2, 2:N], X[:, :, 1, 2:N], X[:, :, 0, 0:N - 2]
                )
                nc.vector.tensor_scalar_mul(X[:, :, 1, 1:N], X[:, :, 1, 1:N], 0.5)
                nc.vector.tensor_scalar_mul(
                    X[:, :, 2, 2:N], X[:, :, 2, 2:N], 1.0 / 3.0
                )

                PA = papool.tile([P, 2, W, XW], f16, tag="PA")
                for bb in range(2):
                    nc.scalar.activation(
                        PA[:, bb, :, 0:Np], X[:, bb, :, 0:Np], EXP, scale=1.0 / TEMP
                    )

                # Qtot = p0 + h + h>>1 + p2>>2, h = p1 + p2; rowsum = sum(p0 + h)
                Hh = agpool.tile([P, 2, XW], f16, tag="H")
                U = agpool.tile([P, 2, L], f16, tag="U")
                Vv = agpool.tile([P, 2, L], f16, tag="V")
                QT = agpool.tile([P, 2, L], f16, tag="QT")
                rs = rspool.tile([P, 2], f32, tag="rs")
                rinv = rspool.tile([P, 2], f32, tag="rinv")
                nc.gpsimd.tensor_add(
                    Hh[:, :, 0:N + 1], PA[:, :, 1, 0:N + 1], PA[:, :, 2, 0:N + 1]
                )
                nc.vector.tensor_add(U[:, :, 0:N], PA[:, :, 0, 0:N], Hh[:, :, 0:N])
                # rowsum via 4x bypass-copy with accumulate (X strip0 is dead)
                for bb in range(2):
                    nc.vector.tensor_scalar(
                        out=X[:, bb, 0, 0:N],
                        in0=U[:, bb, 0:N],
                        scalar1=1.0,
                        scalar2=0.0,
                        op0=MULT,
                        op1=ADD,
                        accum_out=rs[:, bb:bb + 1],
                    )
                nc.gpsimd.tensor_add(
                    Vv[:, :, 0:N], Hh[:, :, 1:N + 1], PA[:, :, 2, 2:Np]
                )
                nc.vector.tensor_add(QT[:, :, 0:N], U[:, :, 0:N], Vv[:, :, 0:N])
                nc.vector.reciprocal(rinv[:], rs[:])
                for bb in range(2):
                    nc.vector.tensor_scalar_mul(
                        QT[:, bb, 0:N], QT[:, bb, 0:N], rinv[:, bb:bb + 1]
                    )

                tq = psum_tr.tile([P, 2, L], f16, tag="tr")
                for bb in range(2):
                    for c in range(j + 1):
                        nc.tensor.transpose(
                            tq[:, bb, c * P:(c + 1) * P],
                            QT[:, bb, c * P:(c + 1) * P],
                            id16[:],
                        )
                qtT = qtpool.tile([P, 2, L], f16, tag="qtT")
                nc.vector.tensor_copy(out=qtT[:, :, 0:N], in_=tq[:, :, 0:N])
                for bb in range(2):
                    outp = outps[bb][j // 2]
                    for c in range(j + 1):
                        nc.tensor.matmul(
                            outp[:, j % 2, :],
                            qtT[:, bb, c * P:(c + 1) * P],
                            vs[bb][:, c, :],
                            start=(c == 0),
                            stop=(c == j),
                        )
                if j % 2 == 1:
                    # both slots of this PSUM half are final: copy + store
                    for bb in range(2):
                        b = 2 * bp + bb
                        osb = opool.tile([P, 2, D], f16, tag=f"osb{bb}")
                        nc.vector.tensor_copy(out=osb[:], in_=outps[bb][j // 2][:])
                        nc.sync.dma_start(
                            out=out_ext[b]
                            .rearrange("(r p) d -> p r d", p=P)[:, j - 1:j + 1, :],
                            in_=osb[:],
                        )
    if not for_sim and not nc.is_finalized():
        nc.finalize()
    return nc


def _numpy_reference(q, key, val, attn_mask):
    # exact port of the reference for non-causal masks (host fallback)
    def area_pool(x, mean):
        b, l, d = x.shape
        cs = np.concatenate([np.zeros((b, 1, d), x.dtype), np.cumsum(x, axis=1)], 1)
        outs = []
        for i in range(W):
            w = i + 1
            s = cs[:, w:, :] - cs[:, :-w, :]
            if mean:
                s = s / np.asarray(w, x.dtype)
            if i > 0:
                s = np.concatenate([np.zeros((b, i, d), x.dtype), s], 1)
            outs.append(s)
        return np.concatenate(outs, 1)

    am = attn_mask[0]
    l = am.shape[0]
    base = np.where(am, -np.inf, np.float32(0.0)).astype(np.float32)
    r = np.arange(l)
    masks = []
    for i in range(W):
        edge = (r[:, None] < i) | (r[None, :] < i)
        masks.append(np.where(edge, -np.inf, base))
    masks = np.concatenate(masks, 1)  # [L, L*W]
    keys = area_pool(key, True)
    allvals = area_pool(val, False)
    ws = np.einsum("bqd,bkd->bqk", q, keys) + masks[None]
    ws = ws / TEMP
    ws = ws - ws.max(-1, keepdims=True)
    e = np.exp(ws)
    wgt = e / e.sum(-1, keepdims=True)
    return np.einsum("bqk,bkd->bqd", wgt, allvals).astype(np.float32)


def _nm16():
    p = np.arange(P)[:, None]
    s = np.arange(P)[None, :]
    return np.where(s > p, np.float16(NEG), np.float16(0.0))


def kernel(q, key, val, attn_mask):
    global LAST_EXEC_NS, LAST_RESULTS
    q = np.asarray(q, dtype=np.float32)
    key = np.asarray(key, dtype=np.float32)
    val = np.asarray(val, dtype=np.float32)
    attn_mask = np.asarray(attn_mask, dtype=bool)

    causal = np.triu(np.ones((L, L), dtype=bool), k=1)[None]
    if not np.array_equal(attn_mask, causal):
        return _numpy_reference(q, key, val, attn_mask)

    from concourse.bass_utils import run_bass_kernel_spmd

    if "nc" not in _NC_CACHE:
        _NC_CACHE["nc"] = _build_nc()
    nc = _NC_CACHE["nc"]

    q16 = np.ascontiguousarray(q.astype(np.float16))
    k16 = np.ascontiguousarray(key.astype(np.float16))
    v16 = np.ascontiguousarray(val.astype(np.float16))
    nm16 = _nm16()
    id16 = np.eye(P, dtype=np.float16)

    in_maps = []
    for i in range(NCORES):
        sl = slice(i * NB, (i + 1) * NB)
        in_maps.append(
            {
                "q": q16[sl],
                "key": k16[sl],
                "val": v16[sl],
                "nm16": nm16,
                "id16": id16,
            }
        )

    res = run_bass_kernel_spmd(nc, in_maps, core_ids=list(range(NCORES)), trace=TRACE)
    LAST_EXEC_NS = getattr(res, "exec_time_ns", None)
    LAST_RESULTS = res
    out = np.concatenate([res.results[i]["out"] for i in range(NCORES)], axis=0)
    return out.astype(np.float32)


# revision 13
# speedup vs baseline: 1.1007x; 1.1007x over previous
"""Area attention (B=64, L=512, D=256, W=3) on 8 TRN2 NeuronCores.

Data parallel over batch: 8 batches per core, processed in pairs so the
elementwise work runs as merged [128, 2, N] instructions (half the
instruction-count overhead). Inputs are cast to f16 on the host (the device
kernel always computed in f16; shipping f16 halves HBM traffic and removes
the on-device casts), and q/k are loaded pre-transposed straight from DRAM
via the XBAR dma-transpose path, which removes all PE input transposes and
their PSUM->SBUF copy-backs.

Per (row-block j, batch pair) the kernel builds the three area-width score
strips [s0, ss1/2, ss2/3] in one SBUF tile so a SINGLE exp activation per
batch (scale 1/T) covers all widths; width scales ride a 4x-mode
tensor_scalar. The softmax denominator is recovered by a 4x tensor_scalar
bypass-copy with accum_out over u = p0 + (p1+p2) (whose row sum IS the
denominator), so no activation accumulator reads are needed. Width pooling
of the values folds into shifted sums of the probabilities (Qtot), keeping
the output matmul contraction at L rather than L*W; 1/rowsum is applied to
Qtot before the PE transpose, so the output matmul result is final and is
DMA'd to DRAM directly from PSUM.
"""

import numpy as np

B, L, D = 64, 512, 256
W = 3
NCORES = 8
NB = B // NCORES  # batches per core
P = 128
RB = L // P  # 4 row blocks of 128
DB = D // P  # 2 contraction blocks of 128
NEG = -30000.0
TEMP = float(np.sqrt(D))  # 16.0
XW = 520  # strip pitch (>= L + 2 pad)

TRACE = False
LAST_EXEC_NS = None
LAST_RESULTS = None

_NC_CACHE = {}


def _build_nc(for_sim=False):
    from contextlib import ExitStack

    import concourse.bacc as bacc
    import concourse.bass as bass
    import concourse.tile as tile
    from concourse import mybir

    f32 = mybir.dt.float32
    f16 = mybir.dt.float16
    EXP = mybir.ActivationFunctionType.Exp
    ADD = mybir.AluOpType.add
    MULT = mybir.AluOpType.mult

    if for_sim:
        nc = bass.Bass()
    else:
        nc = bacc.Bacc(None, target_bir_lowering=False)

    q_ext = nc.declare_dram_parameter("q", [NB, L, D], f16, isOutput=False)
    k_ext = nc.declare_dram_parameter("key", [NB, L, D], f16, isOutput=False)
    v_ext = nc.declare_dram_parameter("val", [NB, L, D], f16, isOutput=False)
    nm16_ext = nc.declare_dram_parameter("nm16", [P, P], f16, isOutput=False)
    id16_ext = nc.declare_dram_parameter("id16", [P, P], f16, isOutput=False)
    out_ext = nc.declare_dram_parameter("out", [NB, L, D], f16, isOutput=True)

    with tile.TileContext(nc) as tc, ExitStack() as ctx:
        const = ctx.enter_context(tc.tile_pool(name="const", bufs=1))
        nm16 = const.tile([P, P], f16)
        id16 = const.tile([P, P], f16)
        warm = const.tile([P, 1], f16)
        nc.gpsimd.dma_start(out=nm16[:], in_=nm16_ext[:])
        nc.gpsimd.dma_start(out=id16[:], in_=id16_ext[:])
        # warm the Exp activation table while the first loads are in flight
        nc.vector.memset(warm[:], 0.0)
        nc.scalar.activation(warm[:], warm[:], EXP, scale=1.0)

        inpool = ctx.enter_context(tc.tile_pool(name="inpool", bufs=2))
        xpool = ctx.enter_context(tc.tile_pool(name="xpool", bufs=2))
        papool = ctx.enter_context(tc.tile_pool(name="papool", bufs=2))
        agpool = ctx.enter_context(tc.tile_pool(name="agpool", bufs=2))
        qtpool = ctx.enter_context(tc.tile_pool(name="qtpool", bufs=2))
        opool = ctx.enter_context(tc.tile_pool(name="opool", bufs=2))
        rspool = ctx.enter_context(tc.tile_pool(name="rspool", bufs=2))
        psum_s0 = ctx.enter_context(tc.tile_pool(name="psum_s0", bufs=1, space="PSUM"))
        psum_tr = ctx.enter_context(tc.tile_pool(name="psum_tr", bufs=2, space="PSUM"))
        psum_o = ctx.enter_context(tc.tile_pool(name="psum_o", bufs=1, space="PSUM"))

        # Software-pipelined emission: for flattened iteration t = (bp, j),
        # the pre-exp stage of t and the post-exp stage of t-1 are emitted
        # together so every engine's in-order queue alternates between two
        # independent dependency chains (no engine idles through the other
        # stages' latency).
        state = {}  # per-pair tiles, keyed by bp

        def emit_pre(bp, j):
            qTs, kTs = state[bp]["qTs"], state[bp]["kTs"]
            N = P * (j + 1)
            Np = N + 2
            # scores for both batches (f32: matmul output requirement)
            s0p = psum_s0.tile([P, 2, L], f32, tag="s0p")
            for bb in range(2):
                for dblk in range(DB):
                    nc.tensor.matmul(
                        s0p[:, bb, 0:N],
                        qTs[bb][:, dblk, j * P:(j + 1) * P],
                        kTs[bb][:, dblk, 0:N],
                        start=(dblk == 0),
                        stop=False,
                    )
                # causal mask accumulated on the PE (identity-stationary)
                nc.tensor.matmul(
                    s0p[:, bb, j * P:N], id16[:], nm16[:], start=False, stop=True
                )

            # strip tile: X[:, b, w, t]; w=0 raw s0, w=1 ss1/2, w=2 ss2/3
            X = xpool.tile([P, 2, W, XW], f16, tag="X")
            nc.gpsimd.memset(X[:, :, 1:3, 0:2], NEG)
            nc.gpsimd.memset(X[:, :, :, N:Np], NEG)
            nc.vector.tensor_copy(out=X[:, :, 0, 0:N], in_=s0p[:, :, 0:N])
            # Pool can't read PSUM: width-2 sum reads the f16 copy
            nc.gpsimd.tensor_add(
                X[:, :, 1, 1:N], X[:, :, 0, 1:N], X[:, :, 0, 0:N - 1]
            )
            nc.vector.tensor_add(
                X[:, :, 2, 2:N], X[:, :, 1, 2:N], X[:, :, 0, 0:N - 2]
            )
            nc.vector.tensor_scalar_mul(X[:, :, 1, 1:N], X[:, :, 1, 1:N], 0.5)
            nc.vector.tensor_scalar_mul(X[:, :, 2, 2:N], X[:, :, 2, 2:N], 1.0 / 3.0)

            PA = papool.tile([P, 2, W, XW], f16, tag="PA")
            for bb in range(2):
                nc.scalar.activation(
                    PA[:, bb, :, 0:Np], X[:, bb, :, 0:Np], EXP, scale=1.0 / TEMP
                )
            return {"bp": bp, "j": j, "X": X, "PA": PA}

        def emit_post(ctx_):
            bp, j, X, PA = ctx_["bp"], ctx_["j"], ctx_["X"], ctx_["PA"]
            st = state[bp]
            vs, outps = st["vs"], st["outps"]
            N = P * (j + 1)
            Np = N + 2
            # Qtot = p0 + h + h>>1 + p2>>2, h = p1 + p2; rowsum = sum(p0 + h)
            Hh = agpool.tile([P, 2, XW], f16, tag="H")
            U = agpool.tile([P, 2, L], f16, tag="U")
            Vv = agpool.tile([P, 2, L], f16, tag="V")
            QT = agpool.tile([P, 2, L], f16, tag="QT")
            rs = rspool.tile([P, 2], f32, tag="rs")
            rinv = rspool.tile([P, 2], f32, tag="rinv")
            nc.gpsimd.tensor_add(
                Hh[:, :, 0:N + 1], PA[:, :, 1, 0:N + 1], PA[:, :, 2, 0:N + 1]
            )
            nc.gpsimd.tensor_add(U[:, :, 0:N], PA[:, :, 0, 0:N], Hh[:, :, 0:N])
            # rowsum via 4x bypass-copy with accumulate (X strip0 is dead)
            for bb in range(2):
                nc.vector.tensor_scalar(
                    out=X[:, bb, 0, 0:N],
                    in0=U[:, bb, 0:N],
                    scalar1=1.0,
                    scalar2=0.0,
                    op0=MULT,
                    op1=ADD,
                    accum_out=rs[:, bb:bb + 1],
                )
            nc.gpsimd.tensor_add(Vv[:, :, 0:N], Hh[:, :, 1:N + 1], PA[:, :, 2, 2:Np])
            nc.gpsimd.tensor_add(QT[:, :, 0:N], U[:, :, 0:N], Vv[:, :, 0:N])
            nc.vector.reciprocal(rinv[:], rs[:])

            # transpose UNNORMALIZED Qtot; 1/rowsum folds into the out-copy
            tq = psum_tr.tile([P, 2, L], f16, tag="tr")
            for bb in range(2):
                for c in range(j + 1):
                    nc.tensor.transpose(
                        tq[:, bb, c * P:(c + 1) * P],
                        QT[:, bb, c * P:(c + 1) * P],
                        id16[:],
                    )
            qtT = qtpool.tile([P, 2, L], f16, tag="qtT")
            nc.vector.tensor_copy(out=qtT[:, :, 0:N], in_=tq[:, :, 0:N])
            for bb in range(2):
                outp = outps[bb][j // 2]
                for c in range(j + 1):
                    nc.tensor.matmul(
                        outp[:, j % 2, :],
                        qtT[:, bb, c * P:(c + 1) * P],
                        vs[bb][:, c, :],
                        start=(c == 0),
                        stop=(c == j),
                    )
            for bb in range(2):
                if j % 2 == 0:
                    osb_t = opool.tile([P, 2, D], f16, tag=f"osb{bb}", name=f"osb{bb}")
                    st["osb"][bb] = osb_t
                # out-copy with 1/rowsum for this slot's 128 query rows
                nc.vector.tensor_scalar_mul(
                    st["osb"][bb][:, j % 2, :],
                    outps[bb][j // 2][:, j % 2, :],
                    rinv[:, bb:bb + 1],
                )
                if j % 2 == 1:
                    b = 2 * bp + bb
                    nc.sync.dma_start(
                        out=out_ext[b]
                        .rearrange("(r p) d -> p r d", p=P)[:, j - 1:j + 1, :],
                        in_=st["osb"][bb][:],
                    )

        pending = None
        for bp in range(NB // 2):
            st = {"qTs": [], "kTs": [], "vs": [], "outps": [], "osb": [None, None]}
            state[bp] = st
            for bb in range(2):
                b = 2 * bp + bb
                qT = inpool.tile([P, DB, L], f16, tag=f"qT{bb}")
                kT = inpool.tile([P, DB, L], f16, tag=f"kT{bb}")
                v16 = inpool.tile([P, RB, D], f16, tag=f"v{bb}")
                nc.sync.dma_start_transpose(out=qT[:], in_=q_ext[b])
                nc.sync.dma_start_transpose(out=kT[:], in_=k_ext[b])
                nc.sync.dma_start(
                    out=v16[:], in_=v_ext[b].rearrange("(r p) d -> p r d", p=P)
                )
                st["qTs"].append(qT)
                st["kTs"].append(kT)
                st["vs"].append(v16)
                outp_a = psum_o.tile([P, 2, D], f32, tag=f"outpa{bb}")
                outp_b = psum_o.tile([P, 2, D], f32, tag=f"outpb{bb}")
                st["outps"].append((outp_a, outp_b))
            for j in range(RB):
                cur = emit_pre(bp, j)
                if pending is not None:
                    emit_post(pending)
                    if pending["bp"] != bp:
                        del state[pending["bp"]]
                pending = cur
        emit_post(pending)
    if not for_sim and not nc.is_finalized():
        nc.finalize()
    return nc


def _numpy_reference(q, key, val, attn_mask):
    # exact port of the reference for non-causal masks (host fallback)
    def area_pool(x, mean):
        b, l, d = x.shape
        cs = np.concatenate([np.zeros((b, 1, d), x.dtype), np.cumsum(x, axis=1)], 1)
        outs = []
        for i in range(W):
            w = i + 1
            s = cs[:, w:, :] - cs[:, :-w, :]
            if mean:
                s = s / np.asarray(w, x.dtype)
            if i > 0:
                s = np.concatenate([np.zeros((b, i, d), x.dtype), s], 1)
            outs.append(s)
        return np.concatenate(outs, 1)

    am = attn_mask[0]
    l = am.shape[0]
    base = np.where(am, -np.inf, np.float32(0.0)).astype(np.float32)
    r = np.arange(l)
    masks = []
    for i in range(W):
        edge = (r[:, None] < i) | (r[None, :] < i)
        masks.append(np.where(edge, -np.inf, base))
    masks = np.concatenate(masks, 1)  # [L, L*W]
    keys = area_pool(key, True)
    allvals = area_pool(val, False)
    ws = np.einsum("bqd,bkd->bqk", q, keys) + masks[None]
    ws = ws / TEMP
    ws = ws - ws.max(-1, keepdims=True)
    e = np.exp(ws)
    wgt = e / e.sum(-1, keepdims=True)
    return np.einsum("bqk,bkd->bqd", wgt, allvals).astype(np.float32)


def _nm16():
    p = np.arange(P)[:, None]
    s = np.arange(P)[None, :]
    return np.where(s > p, np.float16(NEG), np.float16(0.0))


def kernel(q, key, val, attn_mask):
    global LAST_EXEC_NS, LAST_RESULTS
    q = np.asarray(q, dtype=np.float32)
    key = np.asarray(key, dtype=np.float32)
    val = np.asarray(val, dtype=np.float32)
    attn_mask = np.asarray(attn_mask, dtype=bool)

    causal = np.triu(np.ones((L, L), dtype=bool), k=1)[None]
    if not np.array_equal(attn_mask, causal):
        return _numpy_reference(q, key, val, attn_mask)

    from concourse.bass_utils import run_bass_kernel_spmd

    if "nc" not in _NC_CACHE:
        _NC_CACHE["nc"] = _build_nc()
    nc = _NC_CACHE["nc"]

    q16 = np.ascontiguousarray(q.astype(np.float16))
    k16 = np.ascontiguousarray(key.astype(np.float16))
    v16 = np.ascontiguousarray(val.astype(np.float16))
    nm16 = _nm16()
    id16 = np.eye(P, dtype=np.float16)

    in_maps = []
    for i in range(NCORES):
        sl = slice(i * NB, (i + 1) * NB)
        in_maps.append(
            {
                "q": q16[sl],
                "key": k16[sl],
                "val": v16[sl],
                "nm16": nm16,
                "id16": id16,
            }
        )

    res = run_bass_kernel_spmd(nc, in_maps, core_ids=list(range(NCORES)), trace=TRACE)
    LAST_EXEC_NS = getattr(res, "exec_time_ns", None)
    LAST_RESULTS = res
    out = np.concatenate([res.results[i]["out"] for i in range(NCORES)], axis=0)
    return out.astype(np.float32)


# revision 16
# speedup vs baseline: 1.1228x; 1.0201x over previous
"""Area attention (B=64, L=512, D=256, W=3) on 8 TRN2 NeuronCores.

Data parallel over batch: 8 batches per core, processed in pairs so the
elementwise work runs as merged [128, 2, N] instructions (half the
instruction-count overhead). Inputs are cast to f16 on the host (the device
kernel always computed in f16; shipping f16 halves HBM traffic and removes
the on-device casts), and q/k are loaded pre-transposed straight from DRAM
via the XBAR dma-transpose path, which removes all PE input transposes and
their PSUM->SBUF copy-backs.

Per (row-block j, batch pair) the kernel builds the three area-width score
strips [s0, ss1/2, ss2/3] in one SBUF tile so a SINGLE exp activation per
batch (scale 1/T) covers all widths; width scales ride a 4x-mode
tensor_scalar. The softmax denominator is recovered by a 4x tensor_scalar
bypass-copy with accum_out over u = p0 + (p1+p2) (whose row sum IS the
denominator), so no activation accumulator reads are needed. Width pooling
of the values folds into shifted sums of the probabilities (Qtot), keeping
the output matmul contraction at L rather than L*W; 1/rowsum is applied to
Qtot before the PE transpose, so the output matmul result is final and is
DMA'd to DRAM directly from PSUM.
"""

import numpy as np

B, L, D = 64, 512, 256
W = 3
NCORES = 8
NB = B // NCORES  # batches per core
P = 128
RB = L // P  # 4 row blocks of 128
DB = D // P  # 2 contraction blocks of 128
NEG = -30000.0
TEMP = float(np.sqrt(D))  # 16.0
XW = 520  # strip pitch (>= L + 2 pad)

TRACE = False
LAST_EXEC_NS = None
LAST_RESULTS = None

_NC_CACHE = {}


def _build_nc(for_sim=False):
    from contextlib import ExitStack

    import concourse.bacc as bacc
    import concourse.bass as bass
    import concourse.tile as tile
    from concourse import mybir

    f32 = mybir.dt.float32
    f16 = mybir.dt.float16
    EXP = mybir.ActivationFunctionType.Exp
    ADD = mybir.AluOpType.add
    MULT = mybir.AluOpType.mult

    if for_sim:
        nc = bass.Bass()
    else:
        nc = bacc.Bacc(None, target_bir_lowering=False)

    q_ext = nc.declare_dram_parameter("q", [NB, L, D], f16, isOutput=False)
    k_ext = nc.declare_dram_parameter("key", [NB, L, D], f16, isOutput=False)
    v_ext = nc.declare_dram_parameter("val", [NB, L, D], f16, isOutput=False)
    nm16_ext = nc.declare_dram_parameter("nm16", [P, P], f16, isOutput=False)
    id16_ext = nc.declare_dram_parameter("id16", [P, P], f16, isOutput=False)
    out_ext = nc.declare_dram_parameter("out", [NB, L, D], f16, isOutput=True)

    with tile.TileContext(nc) as tc, ExitStack() as ctx:
        const = ctx.enter_context(tc.tile_pool(name="const", bufs=1))
        nm16 = const.tile([P, P], f16)
        id16 = const.tile([P, P], f16)
        warm = const.tile([P, 1], f16)
        nc.gpsimd.dma_start(out=nm16[:], in_=nm16_ext[:])
        nc.gpsimd.dma_start(out=id16[:], in_=id16_ext[:])
        # warm the Exp activation table while the first loads are in flight
        nc.vector.memset(warm[:], 0.0)
        nc.scalar.activation(warm[:], warm[:], EXP, scale=1.0)

        inpool = ctx.enter_context(tc.tile_pool(name="inpool", bufs=2))
        xpool = ctx.enter_context(tc.tile_pool(name="xpool", bufs=2))
        papool = ctx.enter_context(tc.tile_pool(name="papool", bufs=2))
        agpool = ctx.enter_context(tc.tile_pool(name="agpool", bufs=2))
        qtpool = ctx.enter_context(tc.tile_pool(name="qtpool", bufs=2))
        opool = ctx.enter_context(tc.tile_pool(name="opool", bufs=3))
        rspool = ctx.enter_context(tc.tile_pool(name="rspool", bufs=2))
        psum_s0 = ctx.enter_context(tc.tile_pool(name="psum_s0", bufs=1, space="PSUM"))
        psum_tr = ctx.enter_context(tc.tile_pool(name="psum_tr", bufs=2, space="PSUM"))
        psum_o = ctx.enter_context(tc.tile_pool(name="psum_o", bufs=1, space="PSUM"))

        # Software-pipelined emission: for flattened iteration t = (bp, j),
        # the pre-exp stage of t and the post-exp stage of t-1 are emitted
        # together so every engine's in-order queue alternates between two
        # independent dependency chains (no engine idles through the other
        # stages' latency).
        state = {}  # per-pair tiles, keyed by bp

        def emit_pre(bp, j):
            qTs, kTs = state[bp]["qTs"], state[bp]["kTs"]
            N = P * (j + 1)
            Np = N + 2
            # scores for both batches (f32: matmul output requirement)
            s0p = psum_s0.tile([P, 2, L], f32, tag="s0p")
            for bb in range(2):
                for dblk in range(DB):
                    nc.tensor.matmul(
                        s0p[:, bb, 0:N],
                        qTs[bb][:, dblk, j * P:(j + 1) * P],
                        kTs[bb][:, dblk, 0:N],
                        start=(dblk == 0),
                        stop=False,
                    )
                # causal mask accumulated on the PE (identity-stationary)
                nc.tensor.matmul(
                    s0p[:, bb, j * P:N], id16[:], nm16[:], start=False, stop=True
                )

            # strip tile: X[:, b, w, t]; w=0 raw s0, w=1 ss1/2, w=2 ss2/3
            X = xpool.tile([P, 2, W, XW], f16, tag="X")
            nc.gpsimd.memset(X[:, :, 1:3, 0:2], NEG)
            nc.gpsimd.memset(X[:, :, :, N:Np], NEG)
            nc.vector.tensor_copy(out=X[:, :, 0, 0:N], in_=s0p[:, :, 0:N])
            # Pool can't read PSUM: width-2 sum reads the f16 copy
            nc.gpsimd.tensor_add(
                X[:, :, 1, 1:N], X[:, :, 0, 1:N], X[:, :, 0, 0:N - 1]
            )
            nc.vector.tensor_add(
                X[:, :, 2, 2:N], X[:, :, 1, 2:N], X[:, :, 0, 0:N - 2]
            )
            nc.vector.tensor_scalar_mul(X[:, :, 1, 1:N], X[:, :, 1, 1:N], 0.5)
            nc.vector.tensor_scalar_mul(X[:, :, 2, 2:N], X[:, :, 2, 2:N], 1.0 / 3.0)

            PA = papool.tile([P, 2, W, XW], f16, tag="PA")
            for bb in range(2):
                nc.scalar.activation(
                    PA[:, bb, :, 0:Np], X[:, bb, :, 0:Np], EXP, scale=1.0 / TEMP
                )
            return {"bp": bp, "j": j, "X": X, "PA": PA}

        def emit_post(ctx_):
            bp, j, X, PA = ctx_["bp"], ctx_["j"], ctx_["X"], ctx_["PA"]
            st = state[bp]
            vs, outps = st["vs"], st["outps"]
            N = P * (j + 1)
            Np = N + 2
            # Qtot = p0 + h + h>>1 + p2>>2, h = p1 + p2; rowsum = sum(p0 + h)
            Hh = agpool.tile([P, 2, XW], f16, tag="H")
            U = agpool.tile([P, 2, L], f16, tag="U")
            Vv = agpool.tile([P, 2, L], f16, tag="V")
            QT = agpool.tile([P, 2, L], f16, tag="QT")
            rs = rspool.tile([P, 2], f32, tag="rs")
            rinv = rspool.tile([P, 2], f32, tag="rinv")
            nc.gpsimd.tensor_add(
                Hh[:, :, 0:N + 1], PA[:, :, 1, 0:N + 1], PA[:, :, 2, 0:N + 1]
            )
            nc.gpsimd.tensor_add(U[:, :, 0:N], PA[:, :, 0, 0:N], Hh[:, :, 0:N])
            # rowsum via 4x bypass-copy with accumulate (X strip0 is dead)
            for bb in range(2):
                nc.vector.tensor_scalar(
                    out=X[:, bb, 0, 0:N],
                    in0=U[:, bb, 0:N],
                    scalar1=1.0,
                    scalar2=0.0,
                    op0=MULT,
                    op1=ADD,
                    accum_out=rs[:, bb:bb + 1],
                )
            nc.gpsimd.tensor_add(Vv[:, :, 0:N], Hh[:, :, 1:N + 1], PA[:, :, 2, 2:Np])
            nc.gpsimd.tensor_add(QT[:, :, 0:N], U[:, :, 0:N], Vv[:, :, 0:N])
            nc.vector.reciprocal(rinv[:], rs[:])

            # transpose UNNORMALIZED Qtot; 1/rowsum folds into the out-copy
            tq = psum_tr.tile([P, 2, L], f16, tag="tr")
            for bb in range(2):
                for c in range(j + 1):
                    nc.tensor.transpose(
                        tq[:, bb, c * P:(c + 1) * P],
                        QT[:, bb, c * P:(c + 1) * P],
                        id16[:],
                    )
            qtT = qtpool.tile([P, 2, L], f16, tag="qtT")
            nc.vector.tensor_copy(out=qtT[:, :, 0:N], in_=tq[:, :, 0:N])
            for bb in range(2):
                outp = outps[bb][j // 2]
                for c in range(j + 1):
                    nc.tensor.matmul(
                        outp[:, j % 2, :],
                        qtT[:, bb, c * P:(c + 1) * P],
                        vs[bb][:, c, :],
                        start=(c == 0),
                        stop=(c == j),
                    )
            for bb in range(2):
                if j % 2 == 0:
                    osb_t = opool.tile([P, 2, D], f16, tag=f"osb{bb}", name=f"osb{bb}")
                    st["osb"][bb] = osb_t
                # out-copy with 1/rowsum for this slot's 128 query rows
                nc.vector.tensor_scalar_mul(
                    st["osb"][bb][:, j % 2, :],
                    outps[bb][j // 2][:, j % 2, :],
                    rinv[:, bb:bb + 1],
                )
                if j % 2 == 1:
                    b = 2 * bp + bb
                    out_dmas.append(
                        (
                            out_ext[b]
                            .rearrange("(r p) d -> p r d", p=P)[:, j - 1:j + 1, :],
                            st["osb"][bb],
                        )
                    )

        pending = None
        out_dmas = []
        for bp in range(NB // 2):
            st = {"qTs": [], "kTs": [], "vs": [], "outps": [], "osb": [None, None]}
            state[bp] = st
            for bb in range(2):
                b = 2 * bp + bb
                qT = inpool.tile([P, DB, L], f16, tag=f"qT{bb}")
                kT = inpool.tile([P, DB, L], f16, tag=f"kT{bb}")
                nc.sync.dma_start_transpose(out=qT[:], in_=q_ext[b])
                nc.sync.dma_start_transpose(out=kT[:], in_=k_ext[b])
                st["qTs"].append(qT)
                st["kTs"].append(kT)
                outp_a = psum_o.tile([P, 2, D], f32, tag=f"outpa{bb}")
                outp_b = psum_o.tile([P, 2, D], f32, tag=f"outpb{bb}")
                st["outps"].append((outp_a, outp_b))
            for bb in range(2):
                b = 2 * bp + bb
                v16 = inpool.tile([P, RB, D], f16, tag=f"v{bb}")
                nc.sync.dma_start(
                    out=v16[:], in_=v_ext[b].rearrange("(r p) d -> p r d", p=P)
                )
                st["vs"].append(v16)
            for j in range(RB):
                cur = emit_pre(bp, j)
                if pending is not None:
                    emit_post(pending)
                    if pending["bp"] != bp:
                        del state[pending["bp"]]
                pending = cur
                # out-DMAs deferred a full iteration so the SP queue never
                # camps on a semaphore ahead of the next pair's loads
                while len(out_dmas) > 2:
                    dst, src = out_dmas.pop(0)
                    nc.sync.dma_start(out=dst, in_=src[:])
        emit_post(pending)
        for dst, src in out_dmas:
            nc.sync.dma_start(out=dst, in_=src[:])
    if not for_sim and not nc.is_finalized():
        nc.finalize()
    return nc


def _numpy_reference(q, key, val, attn_mask):
    # exact port of the reference for non-causal masks (host fallback)
    def area_pool(x, mean):
        b, l, d = x.shape
        cs = np.concatenate([np.zeros((b, 1, d), x.dtype), np.cumsum(x, axis=1)], 1)
        outs = []
        for i in range(W):
            w = i + 1
            s = cs[:, w:, :] - cs[:, :-w, :]
            if mean:
                s = s / np.asarray(w, x.dtype)
            if i > 0:
                s = np.concatenate([np.zeros((b, i, d), x.dtype), s], 1)
            outs.append(s)
        return np.concatenate(outs, 1)

    am = attn_mask[0]
    l = am.shape[0]
    base = np.where(am, -np.inf, np.float32(0.0)).astype(np.float32)
    r = np.arange(l)
    masks = []
    for i in range(W):
        edge = (r[:, None] < i) | (r[None, :] < i)
        masks.append(np.where(edge, -np.inf, base))
    masks = np.concatenate(masks, 1)  # [L, L*W]
    keys = area_pool(key, True)
    allvals = area_pool(val, False)
    ws = np.einsum("bqd,bkd->bqk", q, keys) + masks[None]
    ws = ws / TEMP
    ws = ws - ws.max(-1, keepdims=True)
    e = np.exp(ws)
    wgt = e / e.sum(-1, keepdims=True)
    return np.einsum("bqk,bkd->bqd", wgt, allvals).astype(np.float32)


def _nm16():
    p = np.arange(P)[:, None]
    s = np.arange(P)[None, :]
    return np.where(s > p, np.float16(NEG), np.float16(0.0))


def kernel(q, key, val, attn_mask):
    global LAST_EXEC_NS, LAST_RESULTS
    q = np.asarray(q, dtype=np.float32)
    key = np.asarray(key, dtype=np.float32)
    val = np.asarray(val, dtype=np.float32)
    attn_mask = np.asarray(attn_mask, dtype=bool)

    causal = np.triu(np.ones((L, L), dtype=bool), k=1)[None]
    if not np.array_equal(attn_mask, causal):
        return _numpy_reference(q, key, val, attn_mask)

    from concourse.bass_utils import run_bass_kernel_spmd

    if "nc" not in _NC_CACHE:
        _NC_CACHE["nc"] = _build_nc()
    nc = _NC_CACHE["nc"]

    q16 = np.ascontiguousarray(q.astype(np.float16))
    k16 = np.ascontiguousarray(key.astype(np.float16))
    v16 = np.ascontiguousarray(val.astype(np.float16))
    nm16 = _nm16()
    id16 = np.eye(P, dtype=np.float16)

    in_maps = []
    for i in range(NCORES):
        sl = slice(i * NB, (i + 1) * NB)
        in_maps.append(
            {
                "q": q16[sl],
                "key": k16[sl],
                "val": v16[sl],
                "nm16": nm16,
                "id16": id16,
            }
        )

    res = run_bass_kernel_spmd(nc, in_maps, core_ids=list(range(NCORES)), trace=TRACE)
    LAST_EXEC_NS = getattr(res, "exec_time_ns", None)
    LAST_RESULTS = res
    out = np.concatenate([res.results[i]["out"] for i in range(NCORES)], axis=0)
    return out.astype(np.float32)


# revision 20
# speedup vs baseline: 1.2130x; 1.0803x over previous
"""Area attention (B=64, L=512, D=256, W=3) on 8 TRN2 NeuronCores.

Data parallel over batch: 8 batches per core, processed in pairs so the
elementwise work runs as merged [128, 2, N] instructions (half the
instruction-count overhead). Inputs are cast to f16 on the host (the device
kernel always computed in f16; shipping f16 halves HBM traffic and removes
the on-device casts), and q/k are loaded pre-transposed straight from DRAM
via the XBAR dma-transpose path, which removes all PE input transposes and
their PSUM->SBUF copy-backs.

Per (row-block j, batch pair) the kernel builds the three area-width score
strips [s0, ss1/2, ss2/3] in one SBUF tile so a SINGLE exp activation per
batch (scale 1/T) covers all widths; width scales ride a 4x-mode
tensor_scalar. The softmax denominator is recovered by a 4x tensor_scalar
bypass-copy with accum_out over u = p0 + (p1+p2) (whose row sum IS the
denominator), so no activation accumulator reads are needed. Width pooling
of the values folds into shifted sums of the probabilities (Qtot), keeping
the output matmul contraction at L rather than L*W; 1/rowsum is applied to
Qtot before the PE transpose, so the output matmul result is final and is
DMA'd to DRAM directly from PSUM.
"""

import numpy as np

B, L, D = 64, 512, 256
W = 3
NCORES = 8
NB = B // NCORES  # batches per core
P = 128
RB = L // P  # 4 row blocks of 128
DB = D // P  # 2 contraction blocks of 128
NEG = -30000.0
TEMP = float(np.sqrt(D))  # 16.0
XW = 520  # strip pitch (>= L + 2 pad)

TRACE = False
LAST_EXEC_NS = None
LAST_RESULTS = None

_NC_CACHE = {}


def _build_nc(for_sim=False):
    from contextlib import ExitStack

    import concourse.bacc as bacc
    import concourse.bass as bass
    import concourse.tile as tile
    from concourse import mybir

    f32 = mybir.dt.float32
    f16 = mybir.dt.float16
    EXP = mybir.ActivationFunctionType.Exp
    ADD = mybir.AluOpType.add
    MULT = mybir.AluOpType.mult

    if for_sim:
        nc = bass.Bass()
    else:
        nc = bacc.Bacc(None, target_bir_lowering=False)

    q_ext = nc.declare_dram_parameter("q", [NB, L, D], f16, isOutput=False)
    k_ext = nc.declare_dram_parameter("key", [NB, L, D], f16, isOutput=False)
    v_ext = nc.declare_dram_parameter("val", [NB, L, D], f16, isOutput=False)
    nm16_ext = nc.declare_dram_parameter("nm16", [P, P], f16, isOutput=False)
    id16_ext = nc.declare_dram_parameter("id16", [P, P], f16, isOutput=False)
    out_ext = nc.declare_dram_parameter("out", [NB, L, D], f16, isOutput=True)

    with tile.TileContext(nc) as tc, ExitStack() as ctx:
        const = ctx.enter_context(tc.tile_pool(name="const", bufs=1))
        nm16 = const.tile([P, P], f16)
        id16 = const.tile([P, P], f16)
        warm = const.tile([P, 1], f16)
        nc.gpsimd.dma_start(out=nm16[:], in_=nm16_ext[:])
        nc.gpsimd.dma_start(out=id16[:], in_=id16_ext[:])
        # warm the Exp activation table while the first loads are in flight
        nc.vector.memset(warm[:], 0.0)
        nc.scalar.activation(warm[:], warm[:], EXP, scale=1.0)

        inpool = ctx.enter_context(tc.tile_pool(name="inpool", bufs=2))
        xpool = ctx.enter_context(tc.tile_pool(name="xpool", bufs=3))
        papool = ctx.enter_context(tc.tile_pool(name="papool", bufs=3))
        agpool = ctx.enter_context(tc.tile_pool(name="agpool", bufs=3))
        qtpool = ctx.enter_context(tc.tile_pool(name="qtpool", bufs=2))
        opool = ctx.enter_context(tc.tile_pool(name="opool", bufs=3))
        rspool = ctx.enter_context(tc.tile_pool(name="rspool", bufs=3))
        psum_s0 = ctx.enter_context(tc.tile_pool(name="psum_s0", bufs=1, space="PSUM"))
        psum_tr = ctx.enter_context(tc.tile_pool(name="psum_tr", bufs=2, space="PSUM"))
        psum_o = ctx.enter_context(tc.tile_pool(name="psum_o", bufs=1, space="PSUM"))

        # Software-pipelined emission: for flattened iteration t = (bp, j),
        # the pre-exp stage of t and the post-exp stage of t-1 are emitted
        # together so every engine's in-order queue alternates between two
        # independent dependency chains (no engine idles through the other
        # stages' latency).
        state = {}  # per-pair tiles, keyed by bp

        def emit_pre(bp, j):
            qTs, kTs = state[bp]["qTs"], state[bp]["kTs"]
            N = P * (j + 1)
            Np = N + 2
            # scores for both batches (f32: matmul output requirement)
            s0p = psum_s0.tile([P, 2, L], f32, tag="s0p")
            for bb in range(2):
                for dblk in range(DB):
                    nc.tensor.matmul(
                        s0p[:, bb, 0:N],
                        qTs[bb][:, dblk, j * P:(j + 1) * P],
                        kTs[bb][:, dblk, 0:N],
                        start=(dblk == 0),
                        stop=False,
                    )
                # causal mask accumulated on the PE (identity-stationary)
                nc.tensor.matmul(
                    s0p[:, bb, j * P:N], id16[:], nm16[:], start=False, stop=True
                )

            # strip tile: X[:, b, w, t]; w=0 raw s0, w=1 ss1/2, w=2 ss2/3
            X = xpool.tile([P, 2, W, XW], f16, tag="X")
            nc.gpsimd.memset(X[:, :, 1:3, 0:2], NEG)
            nc.gpsimd.memset(X[:, :, :, N:Np], NEG)
            nc.vector.tensor_copy(out=X[:, :, 0, 0:N], in_=s0p[:, :, 0:N])
            # Pool can't read PSUM: width-2 sum reads the f16 copy
            nc.gpsimd.tensor_add(
                X[:, :, 1, 1:N], X[:, :, 0, 1:N], X[:, :, 0, 0:N - 1]
            )
            nc.vector.tensor_add(
                X[:, :, 2, 2:N], X[:, :, 1, 2:N], X[:, :, 0, 0:N - 2]
            )
            nc.vector.tensor_scalar_mul(X[:, :, 1, 1:N], X[:, :, 1, 1:N], 0.5)
            nc.vector.tensor_scalar_mul(X[:, :, 2, 2:N], X[:, :, 2, 2:N], 1.0 / 3.0)

            PA = papool.tile([P, 2, W, XW], f16, tag="PA")
            for bb in range(2):
                nc.scalar.activation(
                    PA[:, bb, :, 0:Np], X[:, bb, :, 0:Np], EXP, scale=1.0 / TEMP
                )
            return {"bp": bp, "j": j, "X": X, "PA": PA}

        def emit_agg(ctx_):
            bp, j, X, PA = ctx_["bp"], ctx_["j"], ctx_["X"], ctx_["PA"]
            N = P * (j + 1)
            Np = N + 2
            # Qtot = p0 + h + h>>1 + p2>>2, h = p1 + p2; rowsum = sum(p0 + h)
            Hh = agpool.tile([P, 2, XW], f16, tag="H")
            U = agpool.tile([P, 2, L], f16, tag="U")
            Vv = agpool.tile([P, 2, L], f16, tag="V")
            QT = agpool.tile([P, 2, L], f16, tag="QT")
            rs = rspool.tile([P, 2], f32, tag="rs")
            rinv = rspool.tile([P, 2], f32, tag="rinv")
            nc.gpsimd.tensor_add(
                Hh[:, :, 0:N + 1], PA[:, :, 1, 0:N + 1], PA[:, :, 2, 0:N + 1]
            )
            nc.gpsimd.tensor_add(U[:, :, 0:N], PA[:, :, 0, 0:N], Hh[:, :, 0:N])
            # rowsum via 4x bypass-copy with accumulate (X strip0 is dead)
            for bb in range(2):
                nc.vector.tensor_scalar(
                    out=X[:, bb, 0, 0:N],
                    in0=U[:, bb, 0:N],
                    scalar1=1.0,
                    scalar2=0.0,
                    op0=MULT,
                    op1=ADD,
                    accum_out=rs[:, bb:bb + 1],
                )
            nc.gpsimd.tensor_add(Vv[:, :, 0:N], Hh[:, :, 1:N + 1], PA[:, :, 2, 2:Np])
            nc.gpsimd.tensor_add(QT[:, :, 0:N], U[:, :, 0:N], Vv[:, :, 0:N])
            nc.vector.reciprocal(rinv[:], rs[:])
            ctx_["QT"] = QT
            ctx_["rinv"] = rinv

        def emit_tail(ctx_):
            bp, j = ctx_["bp"], ctx_["j"]
            QT, rinv = ctx_["QT"], ctx_["rinv"]
            st = state[bp]
            vs, outps = st["vs"], st["outps"]
            N = P * (j + 1)
            # transpose UNNORMALIZED Qtot; 1/rowsum folds into the out-copy
            tq = psum_tr.tile([P, 2, L], f16, tag="tr")
            for bb in range(2):
                for c in range(j + 1):
                    nc.tensor.transpose(
                        tq[:, bb, c * P:(c + 1) * P],
                        QT[:, bb, c * P:(c + 1) * P],
                        id16[:],
                    )
            qtT = qtpool.tile([P, 2, L], f16, tag="qtT")
            nc.vector.tensor_copy(out=qtT[:, :, 0:N], in_=tq[:, :, 0:N])
            for bb in range(2):
                outp = outps[bb][j // 2]
                for c in range(j + 1):
                    nc.tensor.matmul(
                        outp[:, j % 2, :],
                        qtT[:, bb, c * P:(c + 1) * P],
                        vs[bb][:, c, :],
                        start=(c == 0),
                        stop=(c == j),
                    )
            for bb in range(2):
                if j % 2 == 0:
                    osb_t = opool.tile([P, 2, D], f16, tag=f"osb{bb}", name=f"osb{bb}")
                    st["osb"][bb] = osb_t
                # out-copy with 1/rowsum for this slot's 128 query rows
                nc.vector.tensor_scalar_mul(
                    st["osb"][bb][:, j % 2, :],
                    outps[bb][j // 2][:, j % 2, :],
                    rinv[:, bb:bb + 1],
                )
                if j % 2 == 1:
                    b = 2 * bp + bb
                    out_dmas.append(
                        (
                            out_ext[b]
                            .rearrange("(r p) d -> p r d", p=P)[:, j - 1:j + 1, :],
                            st["osb"][bb],
                        )
                    )

        def emit_loads(bp):
            st = {"qTs": [], "kTs": [], "vs": [], "outps": [], "osb": [None, None]}
            state[bp] = st
            for bb in range(2):
                b = 2 * bp + bb
                qT = inpool.tile([P, DB, L], f16, tag=f"qT{bb}")
                kT = inpool.tile([P, DB, L], f16, tag=f"kT{bb}")
                # at startup, split loads over both HWDGE queues so the DGE
                # setup overheads pipeline and batch B arrives sooner
                eng = nc.scalar if (bp == 0 and bb == 1) else nc.sync
                eng.dma_start_transpose(out=qT[:], in_=q_ext[b])
                eng.dma_start_transpose(out=kT[:], in_=k_ext[b])
                st["qTs"].append(qT)
                st["kTs"].append(kT)
                outp_a = psum_o.tile([P, 2, D], f32, tag=f"outpa{bb}")
                outp_b = psum_o.tile([P, 2, D], f32, tag=f"outpb{bb}")
                st["outps"].append((outp_a, outp_b))
            for bb in range(2):
                b = 2 * bp + bb
                v16 = inpool.tile([P, RB, D], f16, tag=f"v{bb}")
                nc.sync.dma_start(
                    out=v16[:], in_=v_ext[b].rearrange("(r p) d -> p r d", p=P)
                )
                st["vs"].append(v16)

        pend1 = None  # iteration t-1: aggregation not yet emitted
        pend2 = None  # iteration t-2: tail not yet emitted
        out_dmas = []
        for bp in range(NB // 2):
            if bp not in state:
                emit_loads(bp)
            for j in range(RB):
                if pend2 is not None:
                    emit_tail(pend2)
                cur = emit_pre(bp, j)
                if pend1 is not None:
                    emit_agg(pend1)
                pend2 = pend1
                pend1 = cur
                # out-DMAs deferred so the SP queue never camps on a
                # semaphore ahead of the next pair's loads
                while len(out_dmas) > 2:
                    dst, src = out_dmas.pop(0)
                    nc.sync.dma_start(out=dst, in_=src[:])
        emit_agg(pend1)
        emit_tail(pend2)
        emit_tail(pend1)
        for dst, src in out_dmas:
            nc.sync.dma_start(out=dst, in_=src[:])
    if not for_sim and not nc.is_finalized():
        nc.finalize()
    return nc


def _numpy_reference(q, key, val, attn_mask):
    # exact port of the reference for non-causal masks (host fallback)
    def area_pool(x, mean):
        b, l, d = x.shape
        cs = np.concatenate([np.zeros((b, 1, d), x.dtype), np.cumsum(x, axis=1)], 1)
        outs = []
        for i in range(W):
            w = i + 1
            s = cs[:, w:, :] - cs[:, :-w, :]
            if mean:
                s = s / np.asarray(w, x.dtype)
            if i > 0:
                s = np.concatenate([np.zeros((b, i, d), x.dtype), s], 1)
            outs.append(s)
        return np.concatenate(outs, 1)

    am = attn_mask[0]
    l = am.shape[0]
    base = np.where(am, -np.inf, np.float32(0.0)).astype(np.float32)
    r = np.arange(l)
    masks = []
    for i in range(W):
        edge = (r[:, None] < i) | (r[None, :] < i)
        masks.append(np.where(edge, -np.inf, base))
    masks = np.concatenate(masks, 1)  # [L, L*W]
    keys = area_pool(key, True)
    allvals = area_pool(val, False)
    ws = np.einsum("bqd,bkd->bqk", q, keys) + masks[None]
    ws = ws / TEMP
    ws = ws - ws.max(-1, keepdims=True)
    e = np.exp(ws)
    wgt = e / e.sum(-1, keepdims=True)
    return np.einsum("bqk,bkd->bqd", wgt, allvals).astype(np.float32)


def _nm16():
    p = np.arange(P)[:, None]
    s = np.arange(P)[None, :]
    return np.where(s > p, np.float16(NEG), np.float16(0.0))


def kernel(q, key, val, attn_mask):
    global LAST_EXEC_NS, LAST_RESULTS
    q = np.asarray(q, dtype=np.float32)
    key = np.asarray(key, dtype=np.float32)
    val = np.asarray(val, dtype=np.float32)
    attn_mask = np.asarray(attn_mask, dtype=bool)

    causal = np.triu(np.ones((L, L), dtype=bool), k=1)[None]
    if not np.array_equal(attn_mask, causal):
        return _numpy_reference(q, key, val, attn_mask)

    from concourse.bass_utils import run_bass_kernel_spmd

    if "nc" not in _NC_CACHE:
        _NC_CACHE["nc"] = _build_nc()
    nc = _NC_CACHE["nc"]

    q16 = np.ascontiguousarray(q.astype(np.float16))
    k16 = np.ascontiguousarray(key.astype(np.float16))
    v16 = np.ascontiguousarray(val.astype(np.float16))
    nm16 = _nm16()
    id16 = np.eye(P, dtype=np.float16)

    in_maps = []
    for i in range(NCORES):
        sl = slice(i * NB, (i + 1) * NB)
        in_maps.append(
            {
                "q": q16[sl],
                "key": k16[sl],
                "val": v16[sl],
                "nm16": nm16,
                "id16": id16,
            }
        )

    res = run_bass_kernel_spmd(nc, in_maps, core_ids=list(range(NCORES)), trace=TRACE)
    LAST_EXEC_NS = getattr(res, "exec_time_ns", None)
    LAST_RESULTS = res
    out = np.concatenate([res.results[i]["out"] for i in range(NCORES)], axis=0)
    return out.astype(np.float32)


# revision 22
# speedup vs baseline: 1.2717x; 1.0484x over previous
"""Area attention (B=64, L=512, D=256, W=3) on 8 TRN2 NeuronCores.

Data parallel over batch: 8 batches per core, processed in pairs so the
elementwise work runs as merged [128, 2, N] instructions (half the
instruction-count overhead). Inputs are cast to f16 on the host (the device
kernel always computed in f16; shipping f16 halves HBM traffic and removes
the on-device casts), and q/k are loaded pre-transposed straight from DRAM
via the XBAR dma-transpose path, which removes all PE input transposes and
their PSUM->SBUF copy-backs.

Per (row-block j, batch pair) the kernel builds the three area-width score
strips [s0, ss1/2, ss2/3] in one SBUF tile so a SINGLE exp activation per
batch (scale 1/T) covers all widths; width scales ride a 4x-mode
tensor_scalar. The softmax denominator is recovered by a 4x tensor_scalar
bypass-copy with accum_out over u = p0 + (p1+p2) (whose row sum IS the
denominator), so no activation accumulator reads are needed. Width pooling
of the values folds into shifted sums of the probabilities (Qtot), keeping
the output matmul contraction at L rather than L*W; 1/rowsum is applied to
Qtot before the PE transpose, so the output matmul result is final and is
DMA'd to DRAM directly from PSUM.
"""

import numpy as np

B, L, D = 64, 512, 256
W = 3
NCORES = 8
NB = B // NCORES  # batches per core
P = 128
RB = L // P  # 4 row blocks of 128
DB = D // P  # 2 contraction blocks of 128
NEG = -30000.0
TEMP = float(np.sqrt(D))  # 16.0
XW = 520  # strip pitch (>= L + 2 pad)

TRACE = False
LAST_EXEC_NS = None
LAST_RESULTS = None

_NC_CACHE = {}


def _build_nc(for_sim=False):
    from contextlib import ExitStack

    import concourse.bacc as bacc
    import concourse.bass as bass
    import concourse.tile as tile
    from concourse import mybir

    f32 = mybir.dt.float32
    f16 = mybir.dt.float16
    EXP = mybir.ActivationFunctionType.Exp
    ADD = mybir.AluOpType.add
    MULT = mybir.AluOpType.mult

    if for_sim:
        nc = bass.Bass()
    else:
        nc = bacc.Bacc(None, target_bir_lowering=False)

    q_ext = nc.declare_dram_parameter("q", [NB, L, D], f16, isOutput=False)
    k_ext = nc.declare_dram_parameter("key", [NB, L, D], f16, isOutput=False)
    v_ext = nc.declare_dram_parameter("val", [NB, L, D], f16, isOutput=False)
    nm16_ext = nc.declare_dram_parameter("nm16", [P, P], f16, isOutput=False)
    id16_ext = nc.declare_dram_parameter("id16", [P, P], f16, isOutput=False)
    out_ext = nc.declare_dram_parameter("out", [NB, L, D], f16, isOutput=True)

    with tile.TileContext(nc) as tc, ExitStack() as ctx:
        const = ctx.enter_context(tc.tile_pool(name="const", bufs=1))
        nm16 = const.tile([P, P], f16)
        id16 = const.tile([P, P], f16)
        warm = const.tile([P, 1], f16)
        nc.sync.dma_start(out=nm16[:], in_=nm16_ext[:])
        nc.sync.dma_start(out=id16[:], in_=id16_ext[:])
        # warm the Exp activation table while the first loads are in flight
        nc.vector.memset(warm[:], 0.0)
        nc.scalar.activation(warm[:], warm[:], EXP, scale=1.0)

        inpool = ctx.enter_context(tc.tile_pool(name="inpool", bufs=3))
        xpool = ctx.enter_context(tc.tile_pool(name="xpool", bufs=3))
        papool = ctx.enter_context(tc.tile_pool(name="papool", bufs=3))
        agpool = ctx.enter_context(tc.tile_pool(name="agpool", bufs=3))
        qtpool = ctx.enter_context(tc.tile_pool(name="qtpool", bufs=2))
        opool = ctx.enter_context(tc.tile_pool(name="opool", bufs=3))
        rspool = ctx.enter_context(tc.tile_pool(name="rspool", bufs=3))
        psum_s0 = ctx.enter_context(tc.tile_pool(name="psum_s0", bufs=1, space="PSUM"))
        psum_tr = ctx.enter_context(tc.tile_pool(name="psum_tr", bufs=2, space="PSUM"))
        psum_o = ctx.enter_context(tc.tile_pool(name="psum_o", bufs=1, space="PSUM"))

        # Software-pipelined emission: for flattened iteration t = (bp, j),
        # the pre-exp stage of t and the post-exp stage of t-1 are emitted
        # together so every engine's in-order queue alternates between two
        # independent dependency chains (no engine idles through the other
        # stages' latency).
        state = {}  # per-pair tiles, keyed by bp

        def emit_pre(bp, j):
            qTs, kTs = state[bp]["qTs"], state[bp]["kTs"]
            N = P * (j + 1)
            Np = N + 2
            # scores for both batches (f32: matmul output requirement)
            s0p = psum_s0.tile([P, 2, L], f32, tag="s0p")
            for bb in range(2):
                for dblk in range(DB):
                    nc.tensor.matmul(
                        s0p[:, bb, 0:N],
                        qTs[bb][:, dblk, j * P:(j + 1) * P],
                        kTs[bb][:, dblk, 0:N],
                        start=(dblk == 0),
                        stop=False,
                    )
                # causal mask accumulated on the PE (identity-stationary)
                nc.tensor.matmul(
                    s0p[:, bb, j * P:N], id16[:], nm16[:], start=False, stop=True
                )

            # strip tile: X[:, b, w, t]; w=0 raw s0, w=1 ss1/2, w=2 ss2/3
            X = xpool.tile([P, 2, W, XW], f16, tag="X")
            nc.gpsimd.memset(X[:, :, 1:3, 0:2], NEG)
            nc.gpsimd.memset(X[:, :, :, N:Np], NEG)
            nc.vector.tensor_copy(out=X[:, :, 0, 0:N], in_=s0p[:, :, 0:N])
            # Pool can't read PSUM: width-2 sum reads the f16 copy
            nc.gpsimd.tensor_add(
                X[:, :, 1, 1:N], X[:, :, 0, 1:N], X[:, :, 0, 0:N - 1]
            )
            nc.vector.tensor_add(
                X[:, :, 2, 2:N], X[:, :, 1, 2:N], X[:, :, 0, 0:N - 2]
            )
            nc.vector.tensor_scalar_mul(X[:, :, 1, 1:N], X[:, :, 1, 1:N], 0.5)
            nc.vector.tensor_scalar_mul(X[:, :, 2, 2:N], X[:, :, 2, 2:N], 1.0 / 3.0)

            PA = papool.tile([P, 2, W, XW], f16, tag="PA")
            for bb in range(2):
                nc.scalar.activation(
                    PA[:, bb, :, 0:Np], X[:, bb, :, 0:Np], EXP, scale=1.0 / TEMP
                )
            return {"bp": bp, "j": j, "X": X, "PA": PA}

        def emit_agg(ctx_):
            bp, j, X, PA = ctx_["bp"], ctx_["j"], ctx_["X"], ctx_["PA"]
            N = P * (j + 1)
            Np = N + 2
            # Qtot = p0 + h + h>>1 + p2>>2, h = p1 + p2; rowsum = sum(p0 + h)
            Hh = agpool.tile([P, 2, XW], f16, tag="H")
            U = agpool.tile([P, 2, L], f16, tag="U")
            Vv = agpool.tile([P, 2, L], f16, tag="V")
            QT = agpool.tile([P, 2, L], f16, tag="QT")
            rs = rspool.tile([P, 2], f32, tag="rs")
            rinv = rspool.tile([P, 2], f32, tag="rinv")
            nc.gpsimd.tensor_add(
                Hh[:, :, 0:N + 1], PA[:, :, 1, 0:N + 1], PA[:, :, 2, 0:N + 1]
            )
            nc.gpsimd.tensor_add(U[:, :, 0:N], PA[:, :, 0, 0:N], Hh[:, :, 0:N])
            # rowsum via 4x bypass-copy with accumulate (X strip0 is dead)
            for bb in range(2):
                nc.vector.tensor_scalar(
                    out=X[:, bb, 0, 0:N],
                    in0=U[:, bb, 0:N],
                    scalar1=1.0,
                    scalar2=0.0,
                    op0=MULT,
                    op1=ADD,
                    accum_out=rs[:, bb:bb + 1],
                )
            nc.gpsimd.tensor_add(Vv[:, :, 0:N], Hh[:, :, 1:N + 1], PA[:, :, 2, 2:Np])
            nc.gpsimd.tensor_add(QT[:, :, 0:N], U[:, :, 0:N], Vv[:, :, 0:N])
            nc.vector.reciprocal(rinv[:], rs[:])
            ctx_["QT"] = QT
            ctx_["rinv"] = rinv

        def emit_tail(ctx_):
            bp, j = ctx_["bp"], ctx_["j"]
            QT, rinv = ctx_["QT"], ctx_["rinv"]
            st = state[bp]
            vs, outps = st["vs"], st["outps"]
            N = P * (j + 1)
            # transpose UNNORMALIZED Qtot; 1/rowsum folds into the out-copy
            tq = psum_tr.tile([P, 2, L], f16, tag="tr")
            for bb in range(2):
                for c in range(j + 1):
                    nc.tensor.transpose(
                        tq[:, bb, c * P:(c + 1) * P],
                        QT[:, bb, c * P:(c + 1) * P],
                        id16[:],
                    )
            qtT = qtpool.tile([P, 2, L], f16, tag="qtT")
            nc.vector.tensor_copy(out=qtT[:, :, 0:N], in_=tq[:, :, 0:N])
            for bb in range(2):
                outp = outps[bb][j // 2]
                for c in range(j + 1):
                    nc.tensor.matmul(
                        outp[:, j % 2, :],
                        qtT[:, bb, c * P:(c + 1) * P],
                        vs[bb][:, c, :],
                        start=(c == 0),
                        stop=(c == j),
                    )
            for bb in range(2):
                if j % 2 == 0:
                    osb_t = opool.tile([P, 2, D], f16, tag=f"osb{bb}", name=f"osb{bb}")
                    st["osb"][bb] = osb_t
                # out-copy with 1/rowsum for this slot's 128 query rows
                nc.vector.tensor_scalar_mul(
                    st["osb"][bb][:, j % 2, :],
                    outps[bb][j // 2][:, j % 2, :],
                    rinv[:, bb:bb + 1],
                )
                if j % 2 == 1:
                    b = 2 * bp + bb
                    out_dmas.append(
                        (
                            out_ext[b]
                            .rearrange("(r p) d -> p r d", p=P)[:, j - 1:j + 1, :],
                            st["osb"][bb],
                        )
                    )

        def emit_loads(bp):
            st = {"qTs": [], "kTs": [], "vs": [], "outps": [], "osb": [None, None]}
            state[bp] = st
            for bb in range(2):
                b = 2 * bp + bb
                qT = inpool.tile([P, DB, L], f16, tag=f"qT{bb}")
                kT = inpool.tile([P, DB, L], f16, tag=f"kT{bb}")
                # at startup, split loads over both HWDGE queues so the DGE
                # setup overheads pipeline and batch B arrives sooner
                eng = nc.scalar if (bp == 0 and bb == 1) else nc.sync
                eng.dma_start_transpose(out=qT[:], in_=q_ext[b])
                eng.dma_start_transpose(out=kT[:], in_=k_ext[b])
                st["qTs"].append(qT)
                st["kTs"].append(kT)
                outp_a = psum_o.tile([P, 2, D], f32, tag=f"outpa{bb}")
                outp_b = psum_o.tile([P, 2, D], f32, tag=f"outpb{bb}")
                st["outps"].append((outp_a, outp_b))
            for bb in range(2):
                b = 2 * bp + bb
                v16 = inpool.tile([P, RB, D], f16, tag=f"v{bb}")
                nc.sync.dma_start(
                    out=v16[:], in_=v_ext[b].rearrange("(r p) d -> p r d", p=P)
                )
                st["vs"].append(v16)

        pend1 = None  # iteration t-1: aggregation not yet emitted
        pend2 = None  # iteration t-2: tail not yet emitted
        out_dmas = []
        for bp in range(NB // 2):
            if bp not in state:
                emit_loads(bp)
            for j in range(RB):
                if pend2 is not None:
                    emit_tail(pend2)
                cur = emit_pre(bp, j)
                if pend1 is not None:
                    emit_agg(pend1)
                pend2 = pend1
                pend1 = cur
                # out-DMAs deferred so the SP queue never camps on a
                # semaphore ahead of the next pair's loads
                while len(out_dmas) > 2:
                    dst, src = out_dmas.pop(0)
                    nc.sync.dma_start(out=dst, in_=src[:])
        emit_agg(pend1)
        emit_tail(pend2)
        emit_tail(pend1)
        for dst, src in out_dmas:
            nc.sync.dma_start(out=dst, in_=src[:])
    if not for_sim and not nc.is_finalized():
        nc.finalize()
    return nc


def _numpy_reference(q, key, val, attn_mask):
    # exact port of the reference for non-causal masks (host fallback)
    def area_pool(x, mean):
        b, l, d = x.shape
        cs = np.concatenate([np.zeros((b, 1, d), x.dtype), np.cumsum(x, axis=1)], 1)
        outs = []
        for i in range(W):
            w = i + 1
            s = cs[:, w:, :] - cs[:, :-w, :]
            if mean:
                s = s / np.asarray(w, x.dtype)
            if i > 0:
                s = np.concatenate([np.zeros((b, i, d), x.dtype), s], 1)
            outs.append(s)
        return np.concatenate(outs, 1)

    am = attn_mask[0]
    l = am.shape[0]
    base = np.where(am, -np.inf, np.float32(0.0)).astype(np.float32)
    r = np.arange(l)
    masks = []
    for i in range(W):
        edge = (r[:, None] < i) | (r[None, :] < i)
        masks.append(np.where(edge, -np.inf, base))
    masks = np.concatenate(masks, 1)  # [L, L*W]
    keys = area_pool(key, True)
    allvals = area_pool(val, False)
    ws = np.einsum("bqd,bkd->bqk", q, keys) + masks[None]
    ws = ws / TEMP
    ws = ws - ws.max(-1, keepdims=True)
    e = np.exp(ws)
    wgt = e / e.sum(-1, keepdims=True)
    return np.einsum("bqk,bkd->bqd", wgt, allvals).astype(np.float32)


def _nm16():
    p = np.arange(P)[:, None]
    s = np.arange(P)[None, :]
    return np.where(s > p, np.float16(NEG), np.float16(0.0))


def kernel(q, key, val, attn_mask):
    global LAST_EXEC_NS, LAST_RESULTS
    q = np.asarray(q, dtype=np.float32)
    key = np.asarray(key, dtype=np.float32)
    val = np.asarray(val, dtype=np.float32)
    attn_mask = np.asarray(attn_mask, dtype=bool)

    causal = np.triu(np.ones((L, L), dtype=bool), k=1)[None]
    if not np.array_equal(attn_mask, causal):
        return _numpy_reference(q, key, val, attn_mask)

    from concourse.bass_utils import run_bass_kernel_spmd

    if "nc" not in _NC_CACHE:
        _NC_CACHE["nc"] = _build_nc()
    nc = _NC_CACHE["nc"]

    q16 = np.ascontiguousarray(q.astype(np.float16))
    k16 = np.ascontiguousarray(key.astype(np.float16))
    v16 = np.ascontiguousarray(val.astype(np.float16))
    nm16 = _nm16()
    id16 = np.eye(P, dtype=np.float16)

    in_maps = []
    for i in range(NCORES):
        sl = slice(i * NB, (i + 1) * NB)
        in_maps.append(
            {
                "q": q16[sl],
                "key": k16[sl],
                "val": v16[sl],
                "nm16": nm16,
                "id16": id16,
            }
        )

    res = run_bass_kernel_spmd(nc, in_maps, core_ids=list(range(NCORES)), trace=TRACE)
    LAST_EXEC_NS = getattr(res, "exec_time_ns", None)
    LAST_RESULTS = res
    out = np.concatenate([res.results[i]["out"] for i in range(NCORES)], axis=0)
    return out.astype(np.float32)


# revision 26
# speedup vs baseline: 1.3057x; 1.0267x over previous
"""Area attention (B=64, L=512, D=256, W=3) on 8 TRN2 NeuronCores.

Data parallel over batch: 8 batches per core, processed in pairs so the
elementwise work runs as merged [128, 2, N] instructions (half the
instruction-count overhead). Inputs are cast to f16 on the host (the device
kernel always computed in f16; shipping f16 halves HBM traffic and removes
the on-device casts), and q/k are loaded pre-transposed straight from DRAM
via the XBAR dma-transpose path, which removes all PE input transposes and
their PSUM->SBUF copy-backs.

Per (row-block j, batch pair) the kernel builds the three area-width score
strips [s0, ss1/2, ss2/3] in one SBUF tile so a SINGLE exp activation per
batch (scale 1/T) covers all widths; width scales ride a 4x-mode
tensor_scalar. The softmax denominator is recovered by a 4x tensor_scalar
bypass-copy with accum_out over u = p0 + (p1+p2) (whose row sum IS the
denominator), so no activation accumulator reads are needed. Width pooling
of the values folds into shifted sums of the probabilities (Qtot), keeping
the output matmul contraction at L rather than L*W; 1/rowsum is applied to
Qtot before the PE transpose, so the output matmul result is final and is
DMA'd to DRAM directly from PSUM.
"""

import numpy as np

B, L, D = 64, 512, 256
W = 3
NCORES = 8
NB = B // NCORES  # batches per core
P = 128
RB = L // P  # 4 row blocks of 128
DB = D // P  # 2 contraction blocks of 128
NEG = -30000.0
TEMP = float(np.sqrt(D))  # 16.0
XW = 520  # strip pitch (>= L + 2 pad)

TRACE = False
LAST_EXEC_NS = None
LAST_RESULTS = None

_NC_CACHE = {}


def _build_nc(for_sim=False):
    from contextlib import ExitStack

    import concourse.bacc as bacc
    import concourse.bass as bass
    import concourse.tile as tile
    from concourse import mybir

    f32 = mybir.dt.float32
    f16 = mybir.dt.float16
    EXP = mybir.ActivationFunctionType.Exp
    ADD = mybir.AluOpType.add
    MULT = mybir.AluOpType.mult

    if for_sim:
        nc = bass.Bass()
    else:
        nc = bacc.Bacc(None, target_bir_lowering=False)

    q_ext = nc.declare_dram_parameter("q", [NB, L, D], f16, isOutput=False)
    k_ext = nc.declare_dram_parameter("key", [NB, L, D], f16, isOutput=False)
    v_ext = nc.declare_dram_parameter("val", [NB, L, D], f16, isOutput=False)
    nm16_ext = nc.declare_dram_parameter("nm16", [P, P], f16, isOutput=False)
    id16_ext = nc.declare_dram_parameter("id16", [P, P], f16, isOutput=False)
    out_ext = nc.declare_dram_parameter("out", [NB, L, D], f16, isOutput=True)

    with tile.TileContext(nc) as tc, ExitStack() as ctx:
        const = ctx.enter_context(tc.tile_pool(name="const", bufs=1))
        nm16 = const.tile([P, P], f16)
        id16 = const.tile([P, P], f16)
        warm = const.tile([P, 1], f16)
        nc.sync.dma_start(out=nm16[:], in_=nm16_ext[:])
        nc.sync.dma_start(out=id16[:], in_=id16_ext[:])

        inpool = ctx.enter_context(tc.tile_pool(name="inpool", bufs=3))
        xpool = ctx.enter_context(tc.tile_pool(name="xpool", bufs=3))
        papool = ctx.enter_context(tc.tile_pool(name="papool", bufs=3))
        agpool = ctx.enter_context(tc.tile_pool(name="agpool", bufs=3))
        qtpool = ctx.enter_context(tc.tile_pool(name="qtpool", bufs=2))
        opool = ctx.enter_context(tc.tile_pool(name="opool", bufs=3))
        rspool = ctx.enter_context(tc.tile_pool(name="rspool", bufs=3))
        psum_s0 = ctx.enter_context(tc.tile_pool(name="psum_s0", bufs=1, space="PSUM"))
        psum_tr = ctx.enter_context(tc.tile_pool(name="psum_tr", bufs=2, space="PSUM"))
        psum_o = ctx.enter_context(tc.tile_pool(name="psum_o", bufs=1, space="PSUM"))

        # Software-pipelined emission: for flattened iteration t = (bp, j),
        # the pre-exp stage of t and the post-exp stage of t-1 are emitted
        # together so every engine's in-order queue alternates between two
        # independent dependency chains (no engine idles through the other
        # stages' latency).
        state = {}  # per-pair tiles, keyed by bp

        def emit_pre(bp, j):
            qTs, kTs = state[bp]["qTs"], state[bp]["kTs"]
            N = P * (j + 1)
            Np = N + 2
            # scores for both batches (f32: matmul output requirement)
            s0p = psum_s0.tile([P, 2, L], f32, tag="s0p")
            for bb in range(2):
                for dblk in range(DB):
                    nc.tensor.matmul(
                        s0p[:, bb, 0:N],
                        qTs[bb][:, dblk, j * P:(j + 1) * P],
                        kTs[bb][:, dblk, 0:N],
                        start=(dblk == 0),
                        stop=False,
                    )
                # causal mask accumulated on the PE (identity-stationary)
                nc.tensor.matmul(
                    s0p[:, bb, j * P:N], id16[:], nm16[:], start=False, stop=True
                )

            # strip tile: X[:, b, w, t]; w=0 raw s0, w=1 ss1/2, w=2 ss2/3
            X = xpool.tile([P, 2, W, XW], f16, tag="X")
            nc.gpsimd.memset(X[:, :, 1:3, 0:2], NEG)
            nc.gpsimd.memset(X[:, :, :, N:Np], NEG)
            nc.vector.tensor_copy(out=X[:, :, 0, 0:N], in_=s0p[:, :, 0:N])
            # Pool can't read PSUM: width-2 sum reads the f16 copy
            nc.gpsimd.tensor_add(
                X[:, :, 1, 1:N], X[:, :, 0, 1:N], X[:, :, 0, 0:N - 1]
            )
            nc.vector.tensor_add(
                X[:, :, 2, 2:N], X[:, :, 1, 2:N], X[:, :, 0, 0:N - 2]
            )
            nc.vector.tensor_scalar_mul(X[:, :, 1, 1:N], X[:, :, 1, 1:N], 0.5)
            nc.vector.tensor_scalar_mul(X[:, :, 2, 2:N], X[:, :, 2, 2:N], 1.0 / 3.0)

            PA = papool.tile([P, 2, W, XW], f16, tag="PA")
            for bb in range(2):
                nc.scalar.activation(
                    PA[:, bb, :, 0:Np], X[:, bb, :, 0:Np], EXP, scale=1.0 / TEMP
                )
            return {"bp": bp, "j": j, "X": X, "PA": PA}

        def emit_agg(ctx_):
            bp, j, X, PA = ctx_["bp"], ctx_["j"], ctx_["X"], ctx_["PA"]
            N = P * (j + 1)
            Np = N + 2
            # Qtot = p0 + h + h>>1 + p2>>2, h = p1 + p2; rowsum = sum(p0 + h)
            Hh = agpool.tile([P, 2, XW], f16, tag="H")
            U = agpool.tile([P, 2, L], f16, tag="U")
            Vv = agpool.tile([P, 2, L], f16, tag="V")
            QT = agpool.tile([P, 2, L], f16, tag="QT")
            rs = rspool.tile([P, 2], f32, tag="rs")
            rinv = rspool.tile([P, 2], f32, tag="rinv")
            nc.gpsimd.tensor_add(
                Hh[:, :, 0:N + 1], PA[:, :, 1, 0:N + 1], PA[:, :, 2, 0:N + 1]
            )
            nc.gpsimd.tensor_add(U[:, :, 0:N], PA[:, :, 0, 0:N], Hh[:, :, 0:N])
            # rowsum via 4x bypass-copy with accumulate (X strip0 is dead)
            for bb in range(2):
                nc.vector.tensor_scalar(
                    out=X[:, bb, 0, 0:N],
                    in0=U[:, bb, 0:N],
                    scalar1=1.0,
                    scalar2=0.0,
                    op0=MULT,
                    op1=ADD,
                    accum_out=rs[:, bb:bb + 1],
                )
            nc.gpsimd.tensor_add(Vv[:, :, 0:N], Hh[:, :, 1:N + 1], PA[:, :, 2, 2:Np])
            nc.gpsimd.tensor_add(QT[:, :, 0:N], U[:, :, 0:N], Vv[:, :, 0:N])
            nc.vector.reciprocal(rinv[:], rs[:])
            ctx_["QT"] = QT
            ctx_["rinv"] = rinv

        def emit_tail(ctx_):
            bp, j = ctx_["bp"], ctx_["j"]
            QT, rinv = ctx_["QT"], ctx_["rinv"]
            st = state[bp]
            vs, outps = st["vs"], st["outps"]
            N = P * (j + 1)
            # transpose UNNORMALIZED Qtot; 1/rowsum folds into the out-copy
            tq = psum_tr.tile([P, 2, L], f16, tag="tr")
            for bb in range(2):
                for c in range(j + 1):
                    nc.tensor.transpose(
                        tq[:, bb, c * P:(c + 1) * P],
                        QT[:, bb, c * P:(c + 1) * P],
                        id16[:],
                    )
            qtT = qtpool.tile([P, 2, L], f16, tag="qtT")
            nc.vector.tensor_copy(out=qtT[:, :, 0:N], in_=tq[:, :, 0:N])
            for bb in range(2):
                outp = outps[bb][j // 2]
                for c in range(j + 1):
                    nc.tensor.matmul(
                        outp[:, j % 2, :],
                        qtT[:, bb, c * P:(c + 1) * P],
                        vs[bb][:, c, :],
                        start=(c == 0),
                        stop=(c == j),
                    )
            for bb in range(2):
                if j % 2 == 0:
                    osb_t = opool.tile([P, 2, D], f16, tag=f"osb{bb}", name=f"osb{bb}")
                    st["osb"][bb] = osb_t
                # out-copy with 1/rowsum for this slot's 128 query rows
                # (batch B's copies run on the Activation engine for balance)
                if bb == 0:
                    nc.vector.tensor_scalar_mul(
                        st["osb"][bb][:, j % 2, :],
                        outps[bb][j // 2][:, j % 2, :],
                        rinv[:, bb:bb + 1],
                    )
                else:
                    nc.scalar.mul(
                        st["osb"][bb][:, j % 2, :],
                        outps[bb][j // 2][:, j % 2, :],
                        rinv[:, bb:bb + 1],
                    )
                if j % 2 == 1:
                    b = 2 * bp + bb
                    out_dmas.append(
                        (
                            out_ext[b]
                            .rearrange("(r p) d -> p r d", p=P)[:, j - 1:j + 1, :],
                            st["osb"][bb],
                        )
                    )

        def emit_loads(bp):
            st = {"qTs": [], "kTs": [], "vs": [], "outps": [], "osb": [None, None]}
            state[bp] = st
            for bb in range(2):
                b = 2 * bp + bb
                qT = inpool.tile([P, DB, L], f16, tag=f"qT{bb}")
                kT = inpool.tile([P, DB, L], f16, tag=f"kT{bb}")
                # at startup, split loads over both HWDGE queues so the DGE
                # setup overheads pipeline and batch B arrives sooner, and
                # land the j=0 row-block first so matmuls start early
                eng = nc.scalar if (bp == 0 and bb == 1) else nc.sync
                if bp == 0:
                    eng.dma_start_transpose(out=qT[:, :, 0:P], in_=q_ext[b][0:P, :])
                    eng.dma_start_transpose(out=kT[:, :, 0:P], in_=k_ext[b][0:P, :])
                    eng.dma_start_transpose(out=qT[:, :, P:L], in_=q_ext[b][P:L, :])
                    eng.dma_start_transpose(out=kT[:, :, P:L], in_=k_ext[b][P:L, :])
                else:
                    eng.dma_start_transpose(out=qT[:], in_=q_ext[b])
                    eng.dma_start_transpose(out=kT[:], in_=k_ext[b])
                st["qTs"].append(qT)
                st["kTs"].append(kT)
                outp_a = psum_o.tile([P, 2, D], f32, tag=f"outpa{bb}")
                outp_b = psum_o.tile([P, 2, D], f32, tag=f"outpb{bb}")
                st["outps"].append((outp_a, outp_b))
            for bb in range(2):
                b = 2 * bp + bb
                v16 = inpool.tile([P, RB, D], f16, tag=f"v{bb}")
                nc.sync.dma_start(
                    out=v16[:], in_=v_ext[b].rearrange("(r p) d -> p r d", p=P)
                )
                st["vs"].append(v16)
            if bp == 0:
                # warm the Exp table behind the startup loads on this queue
                nc.vector.memset(warm[:], 0.0)
                nc.scalar.activation(warm[:], warm[:], EXP, scale=1.0)

        pend1 = None  # iteration t-1: aggregation not yet emitted
        pend2 = None  # iteration t-2: tail not yet emitted
        out_dmas = []
        for bp in range(NB // 2):
            if bp not in state:
                emit_loads(bp)
            for j in range(RB):
                if pend2 is not None:
                    emit_tail(pend2)
                cur = emit_pre(bp, j)
                if pend1 is not None:
                    emit_agg(pend1)
                pend2 = pend1
                pend1 = cur
                # out-DMAs deferred so the SP queue never camps on a
                # semaphore ahead of the next pair's loads
                while len(out_dmas) > 2:
                    dst, src = out_dmas.pop(0)
                    nc.sync.dma_start(out=dst, in_=src[:])
        emit_agg(pend1)
        emit_tail(pend2)
        emit_tail(pend1)
        for dst, src in out_dmas:
            nc.sync.dma_start(out=dst, in_=src[:])
    if not for_sim and not nc.is_finalized():
        nc.finalize()
    return nc


def _numpy_reference(q, key, val, attn_mask):
    # exact port of the reference for non-causal masks (host fallback)
    def area_pool(x, mean):
        b, l, d = x.shape
        cs = np.concatenate([np.zeros((b, 1, d), x.dtype), np.cumsum(x, axis=1)], 1)
        outs = []
        for i in range(W):
            w = i + 1
            s = cs[:, w:, :] - cs[:, :-w, :]
            if mean:
                s = s / np.asarray(w, x.dtype)
            if i > 0:
                s = np.concatenate([np.zeros((b, i, d), x.dtype), s], 1)
            outs.append(s)
        return np.concatenate(outs, 1)

    am = attn_mask[0]
    l = am.shape[0]
    base = np.where(am, -np.inf, np.float32(0.0)).astype(np.float32)
    r = np.arange(l)
    masks = []
    for i in range(W):
        edge = (r[:, None] < i) | (r[None, :] < i)
        masks.append(np.where(edge, -np.inf, base))
    masks = np.concatenate(masks, 1)  # [L, L*W]
    keys = area_pool(key, True)
    allvals = area_pool(val, False)
    ws = np.einsum("bqd,bkd->bqk", q, keys) + masks[None]
    ws = ws / TEMP
    ws = ws - ws.max(-1, keepdims=True)
    e = np.exp(ws)
    wgt = e / e.sum(-1, keepdims=True)
    return np.einsum("bqk,bkd->bqd", wgt, allvals).astype(np.float32)


def _nm16():
    p = np.arange(P)[:, None]
    s = np.arange(P)[None, :]
    return np.where(s > p, np.float16(NEG), np.float16(0.0))


def kernel(q, key, val, attn_mask):
    global LAST_EXEC_NS, LAST_RESULTS
    q = np.asarray(q, dtype=np.float32)
    key = np.asarray(key, dtype=np.float32)
    val = np.asarray(val, dtype=np.float32)
    attn_mask = np.asarray(attn_mask, dtype=bool)

    causal = np.triu(np.ones((L, L), dtype=bool), k=1)[None]
    if not np.array_equal(attn_mask, causal):
        return _numpy_reference(q, key, val, attn_mask)

    from concourse.bass_utils import run_bass_kernel_spmd

    if "nc" not in _NC_CACHE:
        _NC_CACHE["nc"] = _build_nc()
    nc = _NC_CACHE["nc"]

    q16 = np.ascontiguousarray(q.astype(np.float16))
    k16 = np.ascontiguousarray(key.astype(np.float16))
    v16 = np.ascontiguousarray(val.astype(np.float16))
    nm16 = _nm16()
    id16 = np.eye(P, dtype=np.float16)

    in_maps = []
    for i in range(NCORES):
        sl = slice(i * NB, (i + 1) * NB)
        in_maps.append(
            {
                "q": q16[sl],
                "key": k16[sl],
                "val": v16[sl],
                "nm16": nm16,
                "id16": id16,
            }
        )

    res = run_bass_kernel_spmd(nc, in_maps, core_ids=list(range(NCORES)), trace=TRACE)
    LAST_EXEC_NS = getattr(res, "exec_time_ns", None)
    LAST_RESULTS = res
    out = np.concatenate([res.results[i]["out"] for i in range(NCORES)], axis=0)
    return out.astype(np.float32)


# revision 31
# speedup vs baseline: 1.3594x; 1.0412x over previous
"""Area attention (B=64, L=512, D=256, W=3) on 8 TRN2 NeuronCores.

Data parallel over batch: 8 batches per core, processed in pairs so the
elementwise work runs as merged [128, 2, N] instructions (half the
instruction-count overhead). Inputs are cast to f16 on the host (the device
kernel always computed in f16; shipping f16 halves HBM traffic and removes
the on-device casts), and q/k are loaded pre-transposed straight from DRAM
via the XBAR dma-transpose path, which removes all PE input transposes and
their PSUM->SBUF copy-backs.

Per (row-block j, batch pair) the kernel builds the three area-width score
strips [s0, ss1/2, ss2/3] in one SBUF tile so a SINGLE exp activation per
batch (scale 1/T) covers all widths; width scales ride a 4x-mode
tensor_scalar. The softmax denominator is recovered by a 4x tensor_scalar
bypass-copy with accum_out over u = p0 + (p1+p2) (whose row sum IS the
denominator), so no activation accumulator reads are needed. Width pooling
of the values folds into shifted sums of the probabilities (Qtot), keeping
the output matmul contraction at L rather than L*W; 1/rowsum is applied to
Qtot before the PE transpose, so the output matmul result is final and is
DMA'd to DRAM directly from PSUM.
"""

import numpy as np

B, L, D = 64, 512, 256
W = 3
NCORES = 8
NB = B // NCORES  # batches per core
P = 128
RB = L // P  # 4 row blocks of 128
DB = D // P  # 2 contraction blocks of 128
NEG = -30000.0
TEMP = float(np.sqrt(D))  # 16.0
XW = 520  # strip pitch (>= L + 2 pad)

TRACE = False
LAST_EXEC_NS = None
LAST_RESULTS = None

_NC_CACHE = {}


def _build_nc(for_sim=False):
    from contextlib import ExitStack

    import concourse.bacc as bacc
    import concourse.bass as bass
    import concourse.tile as tile
    from concourse import mybir

    f32 = mybir.dt.float32
    f16 = mybir.dt.float16
    EXP = mybir.ActivationFunctionType.Exp
    ADD = mybir.AluOpType.add
    MULT = mybir.AluOpType.mult

    if for_sim:
        nc = bass.Bass()
    else:
        nc = bacc.Bacc(None, target_bir_lowering=False)

    q_ext = nc.declare_dram_parameter("q", [NB, L, D], f16, isOutput=False)
    k_ext = nc.declare_dram_parameter("key", [NB, L, D], f16, isOutput=False)
    v_ext = nc.declare_dram_parameter("val", [NB, L, D], f16, isOutput=False)
    nm16_ext = nc.declare_dram_parameter("nm16", [P, P], f16, isOutput=False)
    id16_ext = nc.declare_dram_parameter("id16", [P, P], f16, isOutput=False)
    out_ext = nc.declare_dram_parameter("out", [NB, L, D], f16, isOutput=True)

    with tile.TileContext(nc) as tc, ExitStack() as ctx:
        const = ctx.enter_context(tc.tile_pool(name="const", bufs=1))
        nm16 = const.tile([P, P], f16)
        id16 = const.tile([P, P], f16)
        warm = const.tile([P, 1], f16)
        nc.sync.dma_start(out=nm16[:], in_=nm16_ext[:])
        nc.sync.dma_start(out=id16[:], in_=id16_ext[:])

        inpool = ctx.enter_context(tc.tile_pool(name="inpool", bufs=3))
        xpool = ctx.enter_context(tc.tile_pool(name="xpool", bufs=3))
        papool = ctx.enter_context(tc.tile_pool(name="papool", bufs=3))
        agpool = ctx.enter_context(tc.tile_pool(name="agpool", bufs=3))
        qtpool = ctx.enter_context(tc.tile_pool(name="qtpool", bufs=2))
        opool = ctx.enter_context(tc.tile_pool(name="opool", bufs=3))
        rspool = ctx.enter_context(tc.tile_pool(name="rspool", bufs=3))
        psum_s0 = ctx.enter_context(tc.tile_pool(name="psum_s0", bufs=1, space="PSUM"))
        psum_tr = ctx.enter_context(tc.tile_pool(name="psum_tr", bufs=2, space="PSUM"))
        psum_o = ctx.enter_context(tc.tile_pool(name="psum_o", bufs=1, space="PSUM"))

        # Software-pipelined emission: for flattened iteration t = (bp, j),
        # the pre-exp stage of t and the post-exp stage of t-1 are emitted
        # together so every engine's in-order queue alternates between two
        # independent dependency chains (no engine idles through the other
        # stages' latency).
        state = {}  # per-pair tiles, keyed by bp

        def emit_pre(bp, j):
            qTs, kTs = state[bp]["qTs"], state[bp]["kTs"]
            N = P * (j + 1)
            Np = N + 2
            # scores for both batches (f32: matmul output requirement)
            s0p = psum_s0.tile([P, 2, L], f32, tag="s0p")
            for bb in range(2):
                for dblk in range(DB):
                    nc.tensor.matmul(
                        s0p[:, bb, 0:N],
                        qTs[bb][:, dblk, j * P:(j + 1) * P],
                        kTs[bb][:, dblk, 0:N],
                        start=(dblk == 0),
                        stop=False,
                    )
                # causal mask accumulated on the PE (identity-stationary)
                nc.tensor.matmul(
                    s0p[:, bb, j * P:N], id16[:], nm16[:], start=False, stop=True
                )

            # strip tile: X[:, b, w, t]; w=0 raw s0, w=1 ss1/2, w=2 ss2/3
            X = xpool.tile([P, 2, W, XW], f16, tag="X")
            nc.gpsimd.memset(X[:, :, 1:3, 0:2], NEG)
            nc.gpsimd.memset(X[:, :, :, N:Np], NEG)
            nc.vector.tensor_copy(out=X[:, :, 0, 0:N], in_=s0p[:, :, 0:N])
            # Pool can't read PSUM: width-2 sum reads the f16 copy
            nc.gpsimd.tensor_add(
                X[:, :, 1, 1:N], X[:, :, 0, 1:N], X[:, :, 0, 0:N - 1]
            )
            nc.vector.tensor_add(
                X[:, :, 2, 2:N], X[:, :, 1, 2:N], X[:, :, 0, 0:N - 2]
            )
            nc.vector.tensor_scalar_mul(X[:, :, 1, 1:N], X[:, :, 1, 1:N], 0.5)
            nc.vector.tensor_scalar_mul(X[:, :, 2, 2:N], X[:, :, 2, 2:N], 1.0 / 3.0)

            PA = papool.tile([P, 2, W, XW], f16, tag="PA")
            for bb in range(2):
                nc.scalar.activation(
                    PA[:, bb, :, 0:Np], X[:, bb, :, 0:Np], EXP, scale=1.0 / TEMP
                )
            return {"bp": bp, "j": j, "X": X, "PA": PA}

        def emit_agg(ctx_):
            bp, j, X, PA = ctx_["bp"], ctx_["j"], ctx_["X"], ctx_["PA"]
            N = P * (j + 1)
            Np = N + 2
            # Qtot = p0 + h + h>>1 + p2>>2, h = p1 + p2; rowsum = sum(p0 + h)
            Hh = agpool.tile([P, 2, XW], f16, tag="H")
            U = agpool.tile([P, 2, L], f16, tag="U")
            Vv = agpool.tile([P, 2, L], f16, tag="V")
            QT = agpool.tile([P, 2, L], f16, tag="QT")
            rs = rspool.tile([P, 2], f32, tag="rs")
            rinv = rspool.tile([P, 2], f32, tag="rinv")
            nc.gpsimd.tensor_add(
                Hh[:, :, 0:N + 1], PA[:, :, 1, 0:N + 1], PA[:, :, 2, 0:N + 1]
            )
            nc.gpsimd.tensor_add(U[:, :, 0:N], PA[:, :, 0, 0:N], Hh[:, :, 0:N])
            # rowsum via 4x bypass-copy with accumulate (X strip0 is dead)
            for bb in range(2):
                nc.vector.tensor_scalar(
                    out=X[:, bb, 0, 0:N],
                    in0=U[:, bb, 0:N],
                    scalar1=1.0,
                    scalar2=0.0,
                    op0=MULT,
                    op1=ADD,
                    accum_out=rs[:, bb:bb + 1],
                )
            nc.gpsimd.tensor_add(Vv[:, :, 0:N], Hh[:, :, 1:N + 1], PA[:, :, 2, 2:Np])
            nc.gpsimd.tensor_add(QT[:, :, 0:N], U[:, :, 0:N], Vv[:, :, 0:N])
            nc.vector.reciprocal(rinv[:], rs[:])
            ctx_["QT"] = QT
            ctx_["rinv"] = rinv

        def emit_tail(ctx_):
            bp, j = ctx_["bp"], ctx_["j"]
            QT, rinv = ctx_["QT"], ctx_["rinv"]
            st = state[bp]
            vs, outps = st["vs"], st["outps"]
            N = P * (j + 1)
            # transpose UNNORMALIZED Qtot; 1/rowsum folds into the out-copy
            tq = psum_tr.tile([P, 2, L], f16, tag="tr")
            for bb in range(2):
                for c in range(j + 1):
                    nc.tensor.transpose(
                        tq[:, bb, c * P:(c + 1) * P],
                        QT[:, bb, c * P:(c + 1) * P],
                        id16[:],
                    )
            qtT = qtpool.tile([P, 2, L], f16, tag="qtT")
            nc.vector.tensor_copy(out=qtT[:, :, 0:N], in_=tq[:, :, 0:N])
            for bb in range(2):
                outp = outps[bb][j // 2]
                for c in range(j + 1):
                    nc.tensor.matmul(
                        outp[:, j % 2, :],
                        qtT[:, bb, c * P:(c + 1) * P],
                        vs[bb][:, c, :],
                        start=(c == 0),
                        stop=(c == j),
                    )
            for bb in range(2):
                if j % 2 == 0:
                    osb_t = opool.tile([P, 2, D], f16, tag=f"osb{bb}", name=f"osb{bb}")
                    st["osb"][bb] = osb_t
                # out-copy with 1/rowsum for this slot's 128 query rows
                # (batch B's copies run on the Activation engine for balance)
                if bb == 0:
                    nc.vector.tensor_scalar_mul(
                        st["osb"][bb][:, j % 2, :],
                        outps[bb][j // 2][:, j % 2, :],
                        rinv[:, bb:bb + 1],
                    )
                else:
                    nc.scalar.mul(
                        st["osb"][bb][:, j % 2, :],
                        outps[bb][j // 2][:, j % 2, :],
                        rinv[:, bb:bb + 1],
                    )
                if j % 2 == 1:
                    b = 2 * bp + bb
                    out_dmas.append(
                        (
                            out_ext[b]
                            .rearrange("(r p) d -> p r d", p=P)[:, j - 1:j + 1, :],
                            st["osb"][bb],
                        )
                    )

        def emit_loads(bp):
            st = {"qTs": [], "kTs": [], "vs": [], "outps": [], "osb": [None, None]}
            state[bp] = st
            for bb in range(2):
                b = 2 * bp + bb
                qT = inpool.tile([P, DB, L], f16, tag=f"qT{bb}")
                kT = inpool.tile([P, DB, L], f16, tag=f"kT{bb}")
                # at startup, split loads over both HWDGE queues so the DGE
                # setup overheads pipeline and batch B arrives sooner, and
                # land the j=0 row-block first so matmuls start early
                eng = nc.scalar if (bp == 0 and bb == 1) else nc.sync
                eng.dma_start_transpose(out=qT[:], in_=q_ext[b])
                eng.dma_start_transpose(out=kT[:], in_=k_ext[b])
                st["qTs"].append(qT)
                st["kTs"].append(kT)
                outp_a = psum_o.tile([P, 2, D], f32, tag=f"outpa{bb}")
                outp_b = psum_o.tile([P, 2, D], f32, tag=f"outpb{bb}")
                st["outps"].append((outp_a, outp_b))
            for bb in range(2):
                b = 2 * bp + bb
                v16 = inpool.tile([P, RB, D], f16, tag=f"v{bb}")
                nc.sync.dma_start(
                    out=v16[:], in_=v_ext[b].rearrange("(r p) d -> p r d", p=P)
                )
                st["vs"].append(v16)
            if bp == 0:
                # warm the Exp table behind the startup loads on this queue
                nc.vector.memset(warm[:], 0.0)
                nc.scalar.activation(warm[:], warm[:], EXP, scale=1.0)

        pend1 = None  # iteration t-1: aggregation not yet emitted
        pend2 = None  # iteration t-2: tail not yet emitted
        out_dmas = []
        for bp in range(NB // 2):
            if bp not in state:
                emit_loads(bp)
            for j in range(RB):
                if pend2 is not None:
                    emit_tail(pend2)
                cur = emit_pre(bp, j)
                if pend1 is not None:
                    emit_agg(pend1)
                pend2 = pend1
                pend1 = cur
                # out-DMAs deferred so the SP queue never camps on a
                # semaphore ahead of the next pair's loads
                while len(out_dmas) > 2:
                    dst, src = out_dmas.pop(0)
                    nc.sync.dma_start(out=dst, in_=src[:])
        emit_agg(pend1)
        emit_tail(pend2)
        emit_tail(pend1)
        for dst, src in out_dmas:
            nc.sync.dma_start(out=dst, in_=src[:])
    if not for_sim and not nc.is_finalized():
        nc.finalize()
    return nc


def _numpy_reference(q, key, val, attn_mask):
    # exact port of the reference for non-causal masks (host fallback)
    def area_pool(x, mean):
        b, l, d = x.shape
        cs = np.concatenate([np.zeros((b, 1, d), x.dtype), np.cumsum(x, axis=1)], 1)
        outs = []
        for i in range(W):
            w = i + 1
            s = cs[:, w:, :] - cs[:, :-w, :]
            if mean:
                s = s / np.asarray(w, x.dtype)
            if i > 0:
                s = np.concatenate([np.zeros((b, i, d), x.dtype), s], 1)
            outs.append(s)
        return np.concatenate(outs, 1)

    am = attn_mask[0]
    l = am.shape[0]
    base = np.where(am, -np.inf, np.float32(0.0)).astype(np.float32)
    r = np.arange(l)
    masks = []
    for i in range(W):
        edge = (r[:, None] < i) | (r[None, :] < i)
        masks.append(np.where(edge, -np.inf, base))
    masks = np.concatenate(masks, 1)  # [L, L*W]
    keys = area_pool(key, True)
    allvals = area_pool(val, False)
    ws = np.einsum("bqd,bkd->bqk", q, keys) + masks[None]
    ws = ws / TEMP
    ws = ws - ws.max(-1, keepdims=True)
    e = np.exp(ws)
    wgt = e / e.sum(-1, keepdims=True)
    return np.einsum("bqk,bkd->bqd", wgt, allvals).astype(np.float32)


def _nm16():
    p = np.arange(P)[:, None]
    s = np.arange(P)[None, :]
    return np.where(s > p, np.float16(NEG), np.float16(0.0))


def kernel(q, key, val, attn_mask):
    global LAST_EXEC_NS, LAST_RESULTS
    q = np.asarray(q, dtype=np.float32)
    key = np.asarray(key, dtype=np.float32)
    val = np.asarray(val, dtype=np.float32)
    attn_mask = np.asarray(attn_mask, dtype=bool)

    causal = np.triu(np.ones((L, L), dtype=bool), k=1)[None]
    if not np.array_equal(attn_mask, causal):
        return _numpy_reference(q, key, val, attn_mask)

    from concourse.bass_utils import run_bass_kernel_spmd

    if "nc" not in _NC_CACHE:
        _NC_CACHE["nc"] = _build_nc()
    nc = _NC_CACHE["nc"]

    q16 = np.ascontiguousarray(q.astype(np.float16))
    k16 = np.ascontiguousarray(key.astype(np.float16))
    v16 = np.ascontiguousarray(val.astype(np.float16))
    nm16 = _nm16()
    id16 = np.eye(P, dtype=np.float16)

    in_maps = []
    for i in range(NCORES):
        sl = slice(i * NB, (i + 1) * NB)
        in_maps.append(
            {
                "q": q16[sl],
                "key": k16[sl],
                "val": v16[sl],
                "nm16": nm16,
                "id16": id16,
            }
        )

    res = run_bass_kernel_spmd(nc, in_maps, core_ids=list(range(NCORES)), trace=TRACE)
    LAST_EXEC_NS = getattr(res, "exec_time_ns", None)
    LAST_RESULTS = res
    out = np.concatenate([res.results[i]["out"] for i in range(NCORES)], axis=0)
    return out.astype(np.float32)


# revision 65
# speedup vs baseline: 1.6213x; 1.1927x over previous
"""Area attention (B=64, L=512, D=256, W=3) on 8 TRN2 NeuronCores.

Data parallel over batch: 8 batches per core, processed in pairs so the
elementwise work runs as merged [128, 2, N] instructions (half the
instruction-count overhead). Inputs are cast to f16 on the host (the device
kernel always computed in f16; shipping f16 halves HBM traffic and removes
the on-device casts), and q/k are loaded pre-transposed straight from DRAM
via the XBAR dma-transpose path, which removes all PE input transposes and
their PSUM->SBUF copy-backs.

Per (row-block j, batch pair) the kernel builds the three area-width score
strips [s0, ss1/2, ss2/3] in one SBUF tile so a SINGLE exp activation per
batch (scale 1/T) covers all widths; width scales ride a 4x-mode
tensor_scalar. The softmax denominator is recovered by a 4x tensor_scalar
bypass-copy with accum_out over u = p0 + (p1+p2) (whose row sum IS the
denominator), so no activation accumulator reads are needed. Width pooling
of the values folds into shifted sums of the probabilities (Qtot), keeping
the output matmul contraction at L rather than L*W; 1/rowsum is applied to
Qtot before the PE transpose, so the output matmul result is final and is
DMA'd to DRAM directly from PSUM.
"""

import numpy as np

B, L, D = 64, 512, 256
W = 3
NCORES = 8
NB = B // NCORES  # batches per core
P = 128
RB = L // P  # 4 row blocks of 128
DB = D // P  # 2 contraction blocks of 128
NEG = -30000.0
TEMP = float(np.sqrt(D))  # 16.0
XW = 520  # strip pitch (>= L + 2 pad)

TRACE = False
LAST_EXEC_NS = None
LAST_RESULTS = None

_NC_CACHE = {}


def _build_nc(for_sim=False):
    from contextlib import ExitStack

    import concourse.bacc as bacc
    import concourse.bass as bass
    import concourse.tile as tile
    from concourse import mybir

    f32 = mybir.dt.float32
    f16 = mybir.dt.float16
    EXP = mybir.ActivationFunctionType.Exp
    ADD = mybir.AluOpType.add
    MULT = mybir.AluOpType.mult

    if for_sim:
        nc = bass.Bass()
    else:
        nc = bacc.Bacc(None, target_bir_lowering=False)

    q_ext = nc.declare_dram_parameter("q", [NB, L, D], f16, isOutput=False)
    k_ext = nc.declare_dram_parameter("key", [NB, L, D], f16, isOutput=False)
    v_ext = nc.declare_dram_parameter("val", [NB, L, D], f16, isOutput=False)
    nm16_ext = nc.declare_dram_parameter("nm16", [P, P], f16, isOutput=False)
    id16_ext = nc.declare_dram_parameter("id16", [P, P], f16, isOutput=False)
    out_ext = nc.declare_dram_parameter("out", [NB, L, D], f16, isOutput=True)

    with tile.TileContext(nc) as tc, ExitStack() as ctx:
        const = ctx.enter_context(tc.tile_pool(name="const", bufs=1))
        nm16 = const.tile([P, P], f16)
        id16 = const.tile([P, P], f16)
        warm = const.tile([P, 1], f16)
        nc.sync.dma_start(out=nm16[:], in_=nm16_ext[:])
        nc.sync.dma_start(out=id16[:], in_=id16_ext[:])

        inpool = ctx.enter_context(tc.tile_pool(name="inpool", bufs=3))
        xpool = ctx.enter_context(tc.tile_pool(name="xpool", bufs=4))
        papool = ctx.enter_context(tc.tile_pool(name="papool", bufs=4))
        agpool = ctx.enter_context(tc.tile_pool(name="agpool", bufs=4))
        qtpool = ctx.enter_context(tc.tile_pool(name="qtpool", bufs=3))
        opool = ctx.enter_context(tc.tile_pool(name="opool", bufs=3))
        rspool = ctx.enter_context(tc.tile_pool(name="rspool", bufs=4))
        psum_s0 = ctx.enter_context(tc.tile_pool(name="psum_s0", bufs=1, space="PSUM"))
        psum_tr = ctx.enter_context(tc.tile_pool(name="psum_tr", bufs=2, space="PSUM"))
        psum_o = ctx.enter_context(tc.tile_pool(name="psum_o", bufs=1, space="PSUM"))

        # Software-pipelined emission: for flattened iteration t = (bp, j),
        # the pre-exp stage of t and the post-exp stage of t-1 are emitted
        # together so every engine's in-order queue alternates between two
        # independent dependency chains (no engine idles through the other
        # stages' latency).
        state = {}  # per-pair tiles, keyed by bp

        def emit_pre(bp, j):
            st = state[bp]
            N = P * (j + 1)
            Np = N + 2
            # scores for both batches (f32: matmul output requirement)
            s0p = psum_s0.tile([P, 2, L], f32, tag="s0p")
            for bb in range(2):
                for dblk in range(DB):
                    first = True
                    for qap, kap, klo in st["blocks"](bb, dblk, j):
                        nc.tensor.matmul(
                            s0p[:, bb, klo:klo + kap.shape[-1]],
                            qap,
                            kap,
                            start=(dblk == 0 and first),
                            stop=False,
                        )
                        first = False
                # causal mask accumulated on the PE (identity-stationary)
                nc.tensor.matmul(
                    s0p[:, bb, j * P:N], id16[:], nm16[:], start=False, stop=True
                )

            # strip tile: X[:, b, w, t]; w=0 raw s0, w=1 ss1/2, w=2 ss2/3
            X = xpool.tile([P, 2, W, XW], f16, tag="X")
            nc.gpsimd.memset(X[:, :, 1:3, 0:2], NEG)
            nc.gpsimd.memset(X[:, :, :, N:Np], NEG)
            nc.vector.tensor_copy(out=X[:, :, 0, 0:N], in_=s0p[:, :, 0:N])
            # Pool can't read PSUM: width-2 sum reads the f16 copy
            nc.gpsimd.tensor_add(
                X[:, :, 1, 1:N], X[:, :, 0, 1:N], X[:, :, 0, 0:N - 1]
            )
            nc.gpsimd.tensor_add(
                X[:, :, 2, 2:N], X[:, :, 1, 2:N], X[:, :, 0, 0:N - 2]
            )
            nc.vector.tensor_scalar_mul(X[:, :, 1, 1:N], X[:, :, 1, 1:N], 0.5)
            nc.vector.tensor_scalar_mul(X[:, :, 2, 2:N], X[:, :, 2, 2:N], 1.0 / 3.0)

            PA = papool.tile([P, 2, W, XW], f16, tag="PA")
            for bb in range(2):
                nc.scalar.activation(
                    PA[:, bb, :, 0:Np], X[:, bb, :, 0:Np], EXP, scale=1.0 / TEMP
                )
            return {"bp": bp, "j": j, "X": X, "PA": PA}

        def emit_agg(ctx_):
            bp, j, X, PA = ctx_["bp"], ctx_["j"], ctx_["X"], ctx_["PA"]
            N = P * (j + 1)
            Np = N + 2
            # Qtot = p0 + h + h>>1 + p2>>2, h = p1 + p2; rowsum = sum(p0 + h)
            Hh = agpool.tile([P, 2, XW], f16, tag="H")
            U = agpool.tile([P, 2, L], f16, tag="U")
            Vv = agpool.tile([P, 2, L], f16, tag="V")
            QT = agpool.tile([P, 2, L], f16, tag="QT")
            rs = rspool.tile([P, 2], f32, tag="rs")
            rinv = rspool.tile([P, 2], f32, tag="rinv")
            nc.gpsimd.tensor_add(
                Hh[:, :, 0:N + 1], PA[:, :, 1, 0:N + 1], PA[:, :, 2, 0:N + 1]
            )
            nc.vector.tensor_add(U[:, :, 0:N], PA[:, :, 0, 0:N], Hh[:, :, 0:N])
            nc.gpsimd.tensor_add(Vv[:, :, 0:N], Hh[:, :, 1:N + 1], PA[:, :, 2, 2:Np])
            nc.gpsimd.tensor_add(QT[:, :, 0:N], U[:, :, 0:N], Vv[:, :, 0:N])
            ctx_["QT"] = QT
            ctx_["U"] = U
            ctx_["X"] = X
            ctx_["rs"] = rs
            ctx_["rinv"] = rinv

        def emit_tail(ctx_):
            bp, j = ctx_["bp"], ctx_["j"]
            QT, rinv = ctx_["QT"], ctx_["rinv"]
            U, X, rs = ctx_["U"], ctx_["X"], ctx_["rs"]
            st = state[bp]
            vs, outps = st["vs"], st["outps"]
            N = P * (j + 1)
            # rowsum via 4x bypass-copy with accumulate (X strip0 is dead);
            # emitted two iterations after the agg stage so it never waits
            for bb in range(2):
                nc.vector.tensor_scalar(
                    out=X[:, bb, 0, 0:N],
                    in0=U[:, bb, 0:N],
                    scalar1=1.0,
                    scalar2=0.0,
                    op0=MULT,
                    op1=ADD,
                    accum_out=rs[:, bb:bb + 1],
                )
            nc.vector.reciprocal(rinv[:], rs[:])
            # transpose UNNORMALIZED Qtot; 1/rowsum folds into the out-copy
            tq = psum_tr.tile([P, 2, L], f16, tag="tr")
            for bb in range(2):
                for c in range(j + 1):
                    nc.tensor.transpose(
                        tq[:, bb, c * P:(c + 1) * P],
                        QT[:, bb, c * P:(c + 1) * P],
                        id16[:],
                    )
            qtT = qtpool.tile([P, 2, L], f16, tag="qtT")
            nc.vector.tensor_copy(out=qtT[:, :, 0:N], in_=tq[:, :, 0:N])
            for bb in range(2):
                outp = outps[bb][j // 2]
                for c in range(j + 1):
                    nc.tensor.matmul(
                        outp[:, j % 2, :],
                        qtT[:, bb, c * P:(c + 1) * P],
                        vs[bb][:, c, :],
                        start=(c == 0),
                        stop=(c == j),
                    )
            for bb in range(2):
                if ctx_["half_first"]:
                    osb_t = opool.tile([P, 2, D], f16, tag=f"osb{bb}", name=f"osb{bb}")
                    st["osb"][bb] = osb_t
                # out-copy with 1/rowsum for this slot's 128 query rows
                # (batch B's copies run on the Activation engine for balance)
                if bb == 0:
                    nc.vector.tensor_scalar_mul(
                        st["osb"][bb][:, j % 2, :],
                        outps[bb][j // 2][:, j % 2, :],
                        rinv[:, bb:bb + 1],
                    )
                else:
                    nc.scalar.mul(
                        st["osb"][bb][:, j % 2, :],
                        outps[bb][j // 2][:, j % 2, :],
                        rinv[:, bb:bb + 1],
                    )
                if not ctx_["half_first"]:
                    b = 2 * bp + bb
                    lo = 2 * (j // 2)
                    out_dmas.append(
                        (
                            out_ext[b]
                            .rearrange("(r p) d -> p r d", p=P)[:, lo:lo + 2, :],
                            st["osb"][bb],
                        )
                    )

        def emit_loads(bp):
            st = {"qTs": [], "kTs": [], "vs": [], "outps": [], "osb": [None, None]}
            state[bp] = st
            # XBAR transposes stay on the sync queue only: issuing them
            # from the scalar queue raced nondeterministically on hw.
            # Pair 0 lands the j=0 row-block first (separate contiguous
            # destination tiles: sliced XBAR destinations corrupt on hw).
            if bp == 0:
                los, his = [], []
                for bb in range(2):
                    b = bb
                    qTl = inpool.tile([P, DB, P], f16, tag=f"qTl{bb}", name="qTl")
                    kTl = inpool.tile([P, DB, P], f16, tag=f"kTl{bb}", name="kTl")
                    nc.sync.dma_start_transpose(out=qTl[:], in_=q_ext[b][0:P, :])
                    nc.sync.dma_start_transpose(out=kTl[:], in_=k_ext[b][0:P, :])
                    los.append((qTl, kTl))
                for bb in range(2):
                    b = bb
                    qTh = inpool.tile([P, DB, L - P], f16, tag=f"qTh{bb}", name="qTh")
                    kTh = inpool.tile([P, DB, L - P], f16, tag=f"kTh{bb}", name="kTh")
                    nc.sync.dma_start_transpose(out=qTh[:], in_=q_ext[b][P:L, :])
                    nc.sync.dma_start_transpose(out=kTh[:], in_=k_ext[b][P:L, :])
                    his.append((qTh, kTh))

                def blocks0(bb, dblk, j):
                    qTl, kTl = los[bb]
                    qTh, kTh = his[bb]
                    qap = (
                        qTl[:, dblk, :]
                        if j == 0
                        else qTh[:, dblk, (j - 1) * P:j * P]
                    )
                    out = [(qap, kTl[:, dblk, :], 0)]
                    if j > 0:
                        out.append((qap, kTh[:, dblk, 0:j * P], P))
                    return out

                st["blocks"] = blocks0
            else:
                qTs, kTs = [], []
                for bb in range(2):
                    b = 2 * bp + bb
                    qT = inpool.tile([P, DB, L], f16, tag=f"qT{bb}")
                    kT = inpool.tile([P, DB, L], f16, tag=f"kT{bb}")
                    nc.sync.dma_start_transpose(out=qT[:], in_=q_ext[b])
                    nc.sync.dma_start_transpose(out=kT[:], in_=k_ext[b])
                    qTs.append(qT)
                    kTs.append(kT)

                def blocksN(bb, dblk, j, qTs=qTs, kTs=kTs):
                    N = P * (j + 1)
                    return [
                        (
                            qTs[bb][:, dblk, j * P:(j + 1) * P],
                            kTs[bb][:, dblk, 0:N],
                            0,
                        )
                    ]

                st["blocks"] = blocksN
            for bb in range(2):
                outp_a = psum_o.tile([P, 2, D], f32, tag=f"outpa{bb}")
                outp_b = psum_o.tile([P, 2, D], f32, tag=f"outpb{bb}")
                st["outps"].append((outp_a, outp_b))
            for bb in range(2):
                b = 2 * bp + bb
                v16 = inpool.tile([P, RB, D], f16, tag=f"v{bb}")
                nc.sync.dma_start(
                    out=v16[:], in_=v_ext[b].rearrange("(r p) d -> p r d", p=P)
                )
                st["vs"].append(v16)
            if bp == 0:
                # warm the Exp table behind the startup loads on this queue
                nc.vector.memset(warm[:], 0.0)
                nc.scalar.activation(warm[:], warm[:], EXP, scale=1.0)

        pend1 = None  # iteration t-1: aggregation not yet emitted
        pend2 = None  # iteration t-2: tail not yet emitted
        out_dmas = []
        for bp in range(NB // 2):
            if bp not in state:
                emit_loads(bp)
            asc = bp == 0
            js = range(RB) if asc else range(RB - 1, -1, -1)
            for j in js:
                if pend2 is not None:
                    emit_tail(pend2)
                cur = emit_pre(bp, j)
                cur["half_first"] = (j % 2 == 0) if asc else (j % 2 == 1)
                if pend1 is not None:
                    emit_agg(pend1)
                pend2 = pend1
                pend1 = cur
                # out-DMAs deferred so the SP queue never camps on a
                # semaphore ahead of the next pair's loads
                while len(out_dmas) > 1:
                    dst, src = out_dmas.pop(0)
                    nc.sync.dma_start(out=dst, in_=src[:])
        emit_agg(pend1)
        emit_tail(pend2)
        while len(out_dmas) > 1:
            dst, src_t = out_dmas.pop(0)
            nc.sync.dma_start(out=dst, in_=src_t[:])
        emit_tail(pend1)
        for dst, src_t in out_dmas:
            nc.sync.dma_start(out=dst, in_=src_t[:])
    if not for_sim and not nc.is_finalized():
        nc.finalize()
    return nc


def _numpy_reference(q, key, val, attn_mask):
    # exact port of the reference for non-causal masks (host fallback)
    def area_pool(x, mean):
        b, l, d = x.shape
        cs = np.concatenate([np.zeros((b, 1, d), x.dtype), np.cumsum(x, axis=1)], 1)
        outs = []
        for i in range(W):
            w = i + 1
            s = cs[:, w:, :] - cs[:, :-w, :]
            if mean:
                s = s / np.asarray(w, x.dtype)
            if i > 0:
                s = np.concatenate([np.zeros((b, i, d), x.dtype), s], 1)
            outs.append(s)
        return np.concatenate(outs, 1)

    am = attn_mask[0]
    l = am.shape[0]
    base = np.where(am, -np.inf, np.float32(0.0)).astype(np.float32)
    r = np.arange(l)
    masks = []
    for i in range(W):
        edge = (r[:, None] < i) | (r[None, :] < i)
        masks.append(np.where(edge, -np.inf, base))
    masks = np.concatenate(masks, 1)  # [L, L*W]
    keys = area_pool(key, True)
    allvals = area_pool(val, False)
    ws = np.einsum("bqd,bkd->bqk", q, keys) + masks[None]
    ws = ws / TEMP
    ws = ws - ws.max(-1, keepdims=True)
    e = np.exp(ws)
    wgt = e / e.sum(-1, keepdims=True)
    return np.einsum("bqk,bkd->bqd", wgt, allvals).astype(np.float32)


def _nm16():
    p = np.arange(P)[:, None]
    s = np.arange(P)[None, :]
    return np.where(s > p, np.float16(NEG), np.float16(0.0))


def kernel(q, key, val, attn_mask):
    global LAST_EXEC_NS, LAST_RESULTS
    q = np.asarray(q, dtype=np.float32)
    key = np.asarray(key, dtype=np.float32)
    val = np.asarray(val, dtype=np.float32)
    attn_mask = np.asarray(attn_mask, dtype=bool)

    causal = np.triu(np.ones((L, L), dtype=bool), k=1)[None]
    if not np.array_equal(attn_mask, causal):
        return _numpy_reference(q, key, val, attn_mask)

    from concourse.bass_utils import run_bass_kernel_spmd

    if "nc" not in _NC_CACHE:
        _NC_CACHE["nc"] = _build_nc()
    nc = _NC_CACHE["nc"]

    q16 = np.ascontiguousarray(q.astype(np.float16))
    k16 = np.ascontiguousarray(key.astype(np.float16))
    v16 = np.ascontiguousarray(val.astype(np.float16))

    nm16 = _nm16()
    id16 = np.eye(P, dtype=np.float16)
    in_maps = []
    for i in range(NCORES):
        sl = slice(i * NB, (i + 1) * NB)
        in_maps.append(
            {"q": q16[sl], "key": k16[sl], "val": v16[sl], "nm16": nm16, "id16": id16}
        )

    res = run_bass_kernel_spmd(nc, in_maps, core_ids=list(range(NCORES)), trace=TRACE)
    LAST_EXEC_NS = getattr(res, "exec_time_ns", None)
    LAST_RESULTS = res
    out = np.concatenate([res.results[i]["out"] for i in range(NCORES)], axis=0)
    return out.astype(np.float32)
